# revision 22
# baseline (speedup 1.0000x reference)
"""Causal self-attention (d_model=1024, n_head=16, seq=4096) on 8 trn2 cores.

Sharding: tensor-parallel over heads (2 heads/core) for QKV + attention,
then an AllToAll re-shards y^T from head-sharded to sequence-sharded, so
each core runs the output projection for seq/8 rows with the full w_proj
(no AllReduce). The host concatenates the 8 row-shards.

Per-core layout (bf16 into the PE, fp32 PSUM accumulation):
  - x^T built via PE identity-matmul transposes (the d_model contraction
    needs x in [c, T] layout for both qkv operands).
  - qkv^T = w_slice.T @ x^T lands directly in [chan, T] layout, so qT/kT
    are exactly the lhsT/rhs of the score matmul (scores^T = K Q^T), and
    V' (normal orientation + a ones column) comes from small PE transposes.
  - softmax without max-subtraction (scores ~ N(0,1): exp cannot overflow
    fp32); the denominator falls out of the y^T matmul as the ones-column
    row; normalization uses exp(-ln(denom)) on ScalarE plus a K=1 matmul
    to broadcast the reciprocal across partitions.
  - causal masking: only lower-triangle k-tiles are computed; diagonal
    tiles are masked by a precomputed 0/1 multiply after the exp.
  - emission is braided: prep for block n+1 (x load/transpose/qkv/V') is
    interleaved between the attention groups of q-block n, under a single
    shared PSUM pool, so PE/ACT/DVE/DMA overlap across phases.
"""

import sys
import types

import numpy as np
import ml_dtypes

D_MODEL = 1024
N_HEAD = 16
SEQ = 4096
N_CORES = 8
D_HEAD = 64
CPC = 128            # channels per core (2 heads x 64)
QB = 512             # attention q-block width
BF16 = ml_dtypes.bfloat16


def _install_compat_patches():
    """Stub antenv.axon_hooks (absent in this container) so
    run_bass_kernel_spmd's trace path degrades instead of ImportError."""
    if "antenv.axon_hooks" not in sys.modules:
        mod = types.ModuleType("antenv.axon_hooks")
        mod.get_axon_ntff_profile_hook = lambda: None
        sys.modules["antenv.axon_hooks"] = mod


def _split_multi_waits(nc):
    """The nix walrus here accepts at most ONE sync-wait per instruction
    (setupSyncWait: 'Too many sync wait commands').  Hoist extra waits onto
    same-engine NoOps inserted immediately before the instruction — engine
    streams execute in program order, so semantics are unchanged."""
    import concourse.mybir as mybir

    n = 0
    for fn in nc.m.functions:
        for bb in fn.blocks:
            insts = bb.instructions
            out = []
            for inst in insts:
                si = getattr(inst, "sync_info", None)
                waits = list(si.on_wait) if si is not None else []
                if len(waits) > 1:
                    si.on_wait.clear()
                    for w in waits[:-1]:
                        n += 1
                        nop = mybir.InstNoOp(name=f"I-WSPLIT{n}", ins=[], outs=[])
                        nop.engine = inst.engine
                        nop.sync_info = mybir.SyncInfo(on_wait=[w], on_update=[])
                        out.append(nop)
                    si.on_wait.append(waits[-1])
                out.append(inst)
            bb.instructions = out


def build_nc(seq=SEQ, use_collective=True, split_waits=True):
    """Build the single-core SPMD program (identical on all 8 cores)."""
    import concourse.bass as bass
    import concourse.mybir as mybir
    from concourse.tile import TileContext

    _install_compat_patches()

    f32 = mybir.dt.float32
    bf16 = mybir.dt.bfloat16
    AFT = mybir.ActivationFunctionType

    nT = seq // 128       # T-tiles
    nQB = seq // QB       # attention q-blocks
    SW = seq // N_CORES   # AllToAll shard width (output rows per core)

    nc = bass.Bass("TRN2", target_bir_lowering=False, debug=False,
                   num_devices=N_CORES)
    x_d = nc.dram_tensor("x", [seq, D_MODEL], f32, kind="ExternalInput").ap()
    wq_d = nc.dram_tensor("w_slice", [D_MODEL, 3 * CPC], f32,
                          kind="ExternalInput").ap()
    wp_d = nc.dram_tensor("w_proj", [D_MODEL, D_MODEL], f32,
                          kind="ExternalInput").ap()
    id_d = nc.dram_tensor("ident", [128, 128], bf16, kind="ExternalInput").ap()
    mk_d = nc.dram_tensor("masks", [4, 128, QB], bf16,
                          kind="ExternalInput").ap()
    out_d = nc.dram_tensor("out", [SW, D_MODEL], f32,
                           kind="ExternalOutput").ap()

    with TileContext(nc) as tc:
        with (
            tc.tile_pool(name="per", bufs=1) as per,
            tc.tile_pool(name="stg", bufs=2) as stg,
            tc.tile_pool(name="dram", bufs=1, space="DRAM") as dram,
        ):
            qT = per.tile([128, seq], bf16)      # [2 heads x 64 d, T]
            kT = per.tile([128, seq], bf16)
            Vp = per.tile([128, nT, 130], bf16)  # V' tiles: [v_h0|1|v_h1|1]
            yn0 = per.tile([64, seq], bf16)      # normalized y^T, head 0
            yn1 = per.tile([64, seq], bf16)
            wqkv = per.tile([128, 8, 3 * CPC], bf16)
            wpj = per.tile([128, 8, D_MODEL], bf16)
            iden = per.tile([128, 128], bf16)
            mks = per.tile([128, 4, QB], bf16)
            ones = per.tile([128, 64], f32)
            a2a_sb = per.tile([128, 8, SW], bf16)

            nc.sync.dma_start(iden[:], id_d[:])
            for m in range(4):
                nc.sync.dma_start(mks[:, m, :], mk_d[m])
            nc.any.memset(ones[:], 1.0)
            nc.any.memset(Vp[:, :, 64:65], 1.0)
            nc.any.memset(Vp[:, :, 129:130], 1.0)

            # (weight staging happens inside the xstg pool below)

            a2a_in = dram.tile([N_CORES * CPC, SW], bf16)
            a2a_out = dram.tile([N_CORES * CPC, SW], bf16)

            # ---- phases 0-2, braided emission ------------------------
            # Engines execute their scheduled streams in static order, so
            # overlap must be built into emission order: the prep work
            # (x-load/transpose/qkv/V') for block n+1 is interleaved chunk-
            # by-chunk between the attention groups of q-block n.  Attention
            # qb=n depends only on qkv blocks 0..n, so each braid is legal.
            # PSUM banks: pA 2x1 + sT 2x2 + yt0 1 + yt1 1 = 8
            with (
                tc.tile_pool(name="xp", bufs=1) as xp,
                tc.tile_pool(name="xstg", bufs=3) as xstg,
                tc.tile_pool(name="ps", bufs=2, space="PSUM") as ps,
            ):
                xT = xp.tile([128, 8, seq], bf16)   # [c-chunk part, chunk, T]

                def wqkv_stage():
                    for k in range(8):
                        wtmp = xstg.tile([128, 3 * CPC], f32, tag="xf",
                                         bufs=3, name=f"wtmp_{k}")
                        nc.sync.dma_start(wtmp[:],
                                          wq_d[128 * k:128 * (k + 1), :])
                        nc.vector.tensor_copy(wqkv[:, k, :], wtmp[:])

                def prep_chunks(n):
                    """Emit-closures for block n: loads, x^T, qkv^T, V'."""
                    state = {}

                    def loads():
                        xbs = []
                        for u in range(4):
                            t = 4 * n + u
                            xf = xstg.tile([128, D_MODEL], f32, tag="xf",
                                           bufs=3, name=f"xf_{t}")
                            nc.sync.dma_start(xf[:],
                                              x_d[128 * t:128 * (t + 1), :])
                            xb = xstg.tile([128, D_MODEL], bf16, tag="xb",
                                           bufs=4, name=f"xb_{t}")
                            nc.vector.tensor_copy(xb[:], xf[:])
                            xbs.append(xb)
                        state["xbs"] = xbs

                    def trans(j):
                        # j indexes (x-tile u = j//2, c-chunk quad a = j%2):
                        # one PSUM tile holds 4 c-chunk transposes of a
                        # single x-tile, so work starts after its one load
                        def emit():
                            u, a = divmod(j, 2)
                            tp = ps.tile([128, 512], f32, tag="pA",
                                         name=f"tp_{n}_{j}")
                            for c in range(4):
                                nc.tensor.matmul(
                                    tp[:, 128 * c:128 * (c + 1)],
                                    state["xbs"][u][:, 128 * (4 * a + c):
                                                    128 * (4 * a + c + 1)],
                                    iden[:], start=True, stop=True)
                            nc.vector.tensor_copy(
                                xT[:, 4 * a:4 * (a + 1),
                                   128 * (4 * n + u):128 * (4 * n + u + 1)],
                                tp[:])
                        return emit

                    def qkv(m):
                        def emit():
                            qp = ps.tile([128, 512], f32, tag="pA",
                                         name=f"qp_{n}_{m}")
                            for k in range(8):
                                nc.tensor.matmul(
                                    qp[:],
                                    wqkv[:, k, 128 * m:128 * (m + 1)],
                                    xT[:, k, 512 * n:512 * (n + 1)],
                                    start=(k == 0), stop=(k == 7))
                            if m == 0:
                                nc.vector.tensor_copy(
                                    qT[:, 512 * n:512 * (n + 1)], qp[:])
                            elif m == 1:
                                nc.vector.tensor_copy(
                                    kT[:, 512 * n:512 * (n + 1)], qp[:])
                            else:
                                vs = xstg.tile([128, 512], bf16, tag="vs",
                                               bufs=2, name=f"vs_{n}")
                                nc.vector.tensor_copy(vs[:], qp[:])
                                state["vs"] = vs
                        return emit

                    def vtr(u):
                        def emit():
                            t = 4 * n + u
                            vs = state["vs"]
                            # separate PSUM tiles per head: PE-write plus
                            # DVE-read of one PSUM bank is a HW fault
                            vp0 = ps.tile([128, 64], f32, tag="pA",
                                          name=f"vp0_{t}")
                            vp1 = ps.tile([128, 64], f32, tag="pA",
                                          name=f"vp1_{t}")
                            nc.tensor.matmul(
                                vp0[:], vs[0:64, 128 * u:128 * (u + 1)],
                                iden[0:64, 0:64], start=True, stop=True)
                            nc.tensor.matmul(
                                vp1[:], vs[64:128, 128 * u:128 * (u + 1)],
                                iden[64:128, 64:128], start=True, stop=True)
                            nc.vector.tensor_copy(Vp[:, t, 0:64], vp0[:])
                            nc.vector.tensor_copy(Vp[:, t, 65:129], vp1[:])
                        return emit

                    return ([loads] + [trans(j) for j in range(8)]
                            + [qkv(m) for m in range(3)]
                            + [vtr(u) for u in range(4)])

                def attention_groups(qb, ytps):
                    nkt = 4 * (qb + 1)

                    def group(g):
                        # diagonal k-tiles (d = kt-4qb >= 0) only attend to
                        # q >= 128d: trim score MM / exp / mask / yT MM to
                        # the valid column range [128d, QB).  q-cols below
                        # that are fully masked and, because kt=0 always
                        # covers the full width with start=True, never read.
                        def off(kt):
                            d = kt - 4 * qb
                            return 128 * d if d >= 0 else 0

                        def emit():
                            # h-inner MM order: consecutive score matmuls use
                            # disjoint PE row-groups (h0 rows 0-63, h1 rows
                            # 64-127) so the 16x32x32-subarray PE overlaps
                            # them (K=64 packing, ~2x on the score matmuls)
                            sps = [ps.tile([128, 2 * QB], f32, tag="sT",
                                           name=f"sp_{qb}_{g}_{h}")
                                   for h in (0, 1)]
                            for u in (0, 1):
                                kt = 2 * g + u
                                o = off(kt)
                                for h in (0, 1):
                                    nc.tensor.matmul(
                                        sps[h][:, QB * u + o:QB * (u + 1)],
                                        kT[64 * h:64 * (h + 1),
                                           128 * kt:128 * (kt + 1)],
                                        qT[64 * h:64 * (h + 1),
                                           QB * qb + o:QB * (qb + 1)],
                                        start=True, stop=True)
                            diag = off(2 * g) > 0 or off(2 * g + 1) > 0
                            for h in (0, 1):
                                pt = stg.tile([128, 2 * QB], bf16, tag="pT",
                                              bufs=3, name=f"pt_{qb}_{g}_{h}")
                                if diag:
                                    for u in (0, 1):
                                        o = off(2 * g + u)
                                        nc.scalar.activation(
                                            pt[:, QB * u + o:QB * (u + 1)],
                                            sps[h][:, QB * u + o:QB * (u + 1)],
                                            AFT.Exp, scale=0.125)
                                else:
                                    nc.scalar.activation(pt[:], sps[h][:],
                                                         AFT.Exp, scale=0.125)
                                for u in (0, 1):
                                    kt = 2 * g + u
                                    d = kt - 4 * qb
                                    o = off(kt)
                                    if d >= 0:
                                        nc.vector.tensor_mul(
                                            pt[:, QB * u + o:QB * (u + 1)],
                                            pt[:, QB * u + o:QB * (u + 1)],
                                            mks[:, d, o:QB])
                                    nc.tensor.matmul(
                                        ytps[h][:, o:QB],
                                        Vp[:, kt, 65 * h:65 * (h + 1)],
                                        pt[:, QB * u + o:QB * (u + 1)],
                                        start=(kt == 0),
                                        stop=(kt == nkt - 1))
                        return emit

                    return [group(g) for g in range(nkt // 2)]

                def normalize(qb, ytps):
                    for h in (0, 1):
                        # one copy frees the PSUM accumulator right away; the
                        # denom -> 1/denom -> broadcast -> scale chain then
                        # runs from SBUF off the critical path.
                        yu = stg.tile([65, 2 * QB], f32, tag="dn", bufs=4,
                                      name=f"yu_{qb}_{h}")
                        nc.vector.tensor_copy(yu[:, 0:QB], ytps[h][:])
                        nc.scalar.activation(yu[64:65, QB:2 * QB],
                                             yu[64:65, 0:QB], AFT.Ln)
                        nc.scalar.activation(yu[64:65, QB:2 * QB],
                                             yu[64:65, QB:2 * QB], AFT.Exp,
                                             scale=-1.0)
                        bcp = ps.tile([64, QB], f32, tag="pA",
                                      name=f"bcp_{qb}_{h}")
                        nc.tensor.matmul(bcp[:], ones[64:65, 0:64],
                                         yu[64:65, QB:2 * QB],
                                         start=True, stop=True)
                        bcs = stg.tile([64, QB], f32, tag="bcs", bufs=2,
                                       name=f"bcs_{qb}_{h}")
                        nc.vector.tensor_copy(bcs[:], bcp[:])
                        yn = yn0 if h == 0 else yn1
                        nc.vector.tensor_mul(yn[:, QB * qb:QB * (qb + 1)],
                                             yu[0:64, 0:QB], bcs[:])
                        if SW == QB:
                            # q-block == shard: stage its AllToAll rows now
                            j = qb
                            r0 = 128 * j + 64 * h
                            nc.sync.dma_start(a2a_in[r0:r0 + 64, :],
                                              yn[:, SW * j:SW * (j + 1)])

                def wpj_chunk(k):
                    def emit():
                        # w_proj staged late (projection tail only) and
                        # braided into the final attention block, which has
                        # no other prep work to overlap with
                        ptmp = xstg.tile([128, D_MODEL], f32, tag="xf",
                                         bufs=3, name=f"ptmp_{k}")
                        nc.sync.dma_start(ptmp[:],
                                          wp_d[128 * k:128 * (k + 1), :])
                        nc.vector.tensor_copy(wpj[:, k, :], ptmp[:])
                    return emit

                p0 = prep_chunks(0)
                p0[0]()           # stage-0 x loads lead the DMA queues
                wqkv_stage()
                for c in p0[1:]:
                    c()
                for n in range(nQB):
                    ytps = [ps.tile([65, QB], f32, tag=f"yt{h}", bufs=1,
                                    name=f"yt{h}_{n}") for h in (0, 1)]
                    if n + 1 < nQB:
                        pend = prep_chunks(n + 1)
                    else:
                        pend = [wpj_chunk(k) for k in range(8)]
                    groups = attention_groups(n, ytps)
                    ci = 0
                    for gi, g in enumerate(groups):
                        g()
                        want = (gi + 1) * len(pend) // len(groups)
                        while ci < want:
                            pend[ci]()
                            ci += 1
                    while ci < len(pend):
                        pend[ci]()
                        ci += 1
                    normalize(n, ytps)

            # ---- phase 3: AllToAll head-shard -> seq-shard ----------------
            if SW != QB:
                for j in range(N_CORES):
                    nc.sync.dma_start(a2a_in[128 * j:128 * j + 64, :],
                                      yn0[:, SW * j:SW * (j + 1)])
                    nc.sync.dma_start(a2a_in[128 * j + 64:128 * (j + 1), :],
                                      yn1[:, SW * j:SW * (j + 1)])
            if use_collective:
                nc.gpsimd.collective_compute(
                    "AllToAll", mybir.AluOpType.bypass,
                    ins=[a2a_in.opt()], outs=[a2a_out.opt()],
                    replica_groups=[list(range(N_CORES))])
            else:
                # timing-model variant (TimelineSim can't simulate
                # collectives): stand-in DRAM->DRAM copy
                nc.sync.dma_start(a2a_out[:], a2a_in[:])
            for j in range(N_CORES):
                nc.sync.dma_start(a2a_sb[:, j, :],
                                  a2a_out[128 * j:128 * (j + 1), :])

            # ---- phase 4: output projection for this core's SW rows -------
            with tc.tile_pool(name="psC", bufs=2, space="PSUM") as psC:
                mw = min(128, SW)
                for m in range(SW // mw):
                    pp = psC.tile([mw, D_MODEL], f32, tag="pp")
                    for n2 in (0, 1):
                        for k in range(8):
                            nc.tensor.matmul(
                                pp[:, 512 * n2:512 * (n2 + 1)],
                                a2a_sb[:, k, mw * m:mw * (m + 1)],
                                wpj[:, k, 512 * n2:512 * (n2 + 1)],
                                start=(k == 0), stop=(k == 7))
                    ob = stg.tile([mw, D_MODEL], f32, tag="ob", bufs=2)
                    nc.vector.tensor_copy(ob[:], pp[:])
                    nc.sync.dma_start(out_d[mw * m:mw * (m + 1), :], ob[:])

    if split_waits:
        _split_multi_waits(nc)
    return nc


def make_aux_inputs():
    ident = np.eye(128, dtype=BF16)
    k_idx = np.arange(128)[:, None]
    q_idx = np.arange(QB)[None, :]
    masks = np.stack(
        [((k_idx + 128 * d) <= q_idx).astype(BF16) for d in range(4)], axis=0)
    return ident, masks


def make_in_maps(x, w_qkv, w_proj, seq=SEQ):
    x = np.asarray(x, dtype=np.float32).reshape(seq, D_MODEL)
    w_qkv = np.asarray(w_qkv, dtype=np.float32)
    w_proj = np.asarray(w_proj, dtype=np.float32)
    ident, masks = make_aux_inputs()
    in_maps = []
    for i in range(N_CORES):
        sl = slice(CPC * i, CPC * (i + 1))
        w_slice = np.concatenate(
            [w_qkv[:, sl], w_qkv[:, D_MODEL:][:, sl],
             w_qkv[:, 2 * D_MODEL:][:, sl]], axis=1)
        in_maps.append({
            "x": x,
            "w_slice": np.ascontiguousarray(w_slice),
            "w_proj": w_proj,
            "ident": ident,
            "masks": masks,
        })
    return in_maps


_NC_CACHE = {}


def kernel(x, w_qkv, w_proj):
    """Full inputs in, full output out. Shards internally across 8 cores."""
    try:
        import os
        import jax
        jax.config.update("jax_compilation_cache_dir",
                          os.path.expanduser("~/.cache/jax_bass_kernel"))
        jax.config.update("jax_persistent_cache_min_compile_time_secs", 0.0)
    except Exception:
        pass
    from concourse.bass_utils import run_bass_kernel_spmd

    x = np.asarray(x, dtype=np.float32)
    batch = x.shape[0]
    seq = x.shape[1]
    if seq not in _NC_CACHE:
        _NC_CACHE[seq] = build_nc(seq)
    nc = _NC_CACHE[seq]
    in_maps = make_in_maps(x, w_qkv, w_proj, seq=seq)
    res = run_bass_kernel_spmd(nc, in_maps, list(range(N_CORES)))
    out = np.concatenate([res.results[j]["out"] for j in range(N_CORES)],
                         axis=0)
    return out.reshape(batch, seq, D_MODEL).astype(np.float32)



# revision 23
# speedup vs baseline: 1.0103x; 1.0103x over previous
"""Causal self-attention (d_model=1024, n_head=16, seq=4096) on 8 trn2 cores.

Sharding: tensor-parallel over heads (2 heads/core) for QKV + attention,
then an AllToAll re-shards y^T from head-sharded to sequence-sharded, so
each core runs the output projection for seq/8 rows with the full w_proj
(no AllReduce). The host concatenates the 8 row-shards.

Per-core layout (bf16 into the PE, fp32 PSUM accumulation):
  - x^T built via PE identity-matmul transposes (the d_model contraction
    needs x in [c, T] layout for both qkv operands).
  - qkv^T = w_slice.T @ x^T lands directly in [chan, T] layout, so qT/kT
    are exactly the lhsT/rhs of the score matmul (scores^T = K Q^T), and
    V' (normal orientation + a ones column) comes from small PE transposes.
  - softmax without max-subtraction (scores ~ N(0,1): exp cannot overflow
    fp32); the denominator falls out of the y^T matmul as the ones-column
    row; normalization uses exp(-ln(denom)) on ScalarE plus a K=1 matmul
    to broadcast the reciprocal across partitions.
  - causal masking: only lower-triangle k-tiles are computed; diagonal
    tiles are masked by a precomputed 0/1 multiply after the exp.
  - emission is braided: prep for block n+1 (x load/transpose/qkv/V') is
    interleaved between the attention groups of q-block n, under a single
    shared PSUM pool, so PE/ACT/DVE/DMA overlap across phases.
"""

import sys
import types

import numpy as np
import ml_dtypes

D_MODEL = 1024
N_HEAD = 16
SEQ = 4096
N_CORES = 8
D_HEAD = 64
CPC = 128            # channels per core (2 heads x 64)
QB = 512             # attention q-block width
BF16 = ml_dtypes.bfloat16


def _install_compat_patches():
    """Stub antenv.axon_hooks (absent in this container) so
    run_bass_kernel_spmd's trace path degrades instead of ImportError."""
    if "antenv.axon_hooks" not in sys.modules:
        mod = types.ModuleType("antenv.axon_hooks")
        mod.get_axon_ntff_profile_hook = lambda: None
        sys.modules["antenv.axon_hooks"] = mod


def _split_multi_waits(nc):
    """The nix walrus here accepts at most ONE sync-wait per instruction
    (setupSyncWait: 'Too many sync wait commands').  Hoist extra waits onto
    same-engine NoOps inserted immediately before the instruction — engine
    streams execute in program order, so semantics are unchanged."""
    import concourse.mybir as mybir

    n = 0
    for fn in nc.m.functions:
        for bb in fn.blocks:
            insts = bb.instructions
            out = []
            for inst in insts:
                si = getattr(inst, "sync_info", None)
                waits = list(si.on_wait) if si is not None else []
                if len(waits) > 1:
                    si.on_wait.clear()
                    for w in waits[:-1]:
                        n += 1
                        nop = mybir.InstNoOp(name=f"I-WSPLIT{n}", ins=[], outs=[])
                        nop.engine = inst.engine
                        nop.sync_info = mybir.SyncInfo(on_wait=[w], on_update=[])
                        out.append(nop)
                    si.on_wait.append(waits[-1])
                out.append(inst)
            bb.instructions = out


def build_nc(seq=SEQ, use_collective=True, split_waits=True):
    """Build the single-core SPMD program (identical on all 8 cores)."""
    import concourse.bass as bass
    import concourse.mybir as mybir
    from concourse.tile import TileContext

    _install_compat_patches()

    f32 = mybir.dt.float32
    bf16 = mybir.dt.bfloat16
    AFT = mybir.ActivationFunctionType

    nT = seq // 128       # T-tiles
    nQB = seq // QB       # attention q-blocks
    SW = seq // N_CORES   # AllToAll shard width (output rows per core)

    nc = bass.Bass("TRN2", target_bir_lowering=False, debug=False,
                   num_devices=N_CORES)
    x_d = nc.dram_tensor("x", [seq, D_MODEL], f32, kind="ExternalInput").ap()
    wq_d = nc.dram_tensor("w_slice", [D_MODEL, 3 * CPC], f32,
                          kind="ExternalInput").ap()
    wp_d = nc.dram_tensor("w_proj", [D_MODEL, D_MODEL], f32,
                          kind="ExternalInput").ap()
    id_d = nc.dram_tensor("ident", [128, 128], bf16, kind="ExternalInput").ap()
    mk_d = nc.dram_tensor("masks", [4, 128, QB], bf16,
                          kind="ExternalInput").ap()
    out_d = nc.dram_tensor("out", [SW, D_MODEL], f32,
                           kind="ExternalOutput").ap()

    with TileContext(nc) as tc:
        with (
            tc.tile_pool(name="per", bufs=1) as per,
            tc.tile_pool(name="stg", bufs=2) as stg,
            tc.tile_pool(name="dram", bufs=1, space="DRAM") as dram,
        ):
            qT = per.tile([128, seq], bf16)      # [2 heads x 64 d, T]
            kT = per.tile([128, seq], bf16)
            Vp = per.tile([128, nT, 130], bf16)  # V' tiles: [v_h0|1|v_h1|1]
            yn0 = per.tile([64, seq], bf16)      # normalized y^T, head 0
            yn1 = per.tile([64, seq], bf16)
            wqkv = per.tile([128, 8, 3 * CPC], bf16)
            wpj = per.tile([128, 8, D_MODEL], bf16)
            iden = per.tile([128, 128], bf16)
            mks = per.tile([128, 4, QB], bf16)
            ones = per.tile([128, 64], f32)
            a2a_sb = per.tile([128, 8, SW], bf16)

            nc.sync.dma_start(iden[:], id_d[:])
            for m in range(4):
                nc.sync.dma_start(mks[:, m, :], mk_d[m])
            nc.any.memset(ones[:], 1.0)
            nc.any.memset(Vp[:, :, 64:65], 1.0)
            nc.any.memset(Vp[:, :, 129:130], 1.0)

            # (weight staging happens inside the xstg pool below)

            a2a_in = dram.tile([N_CORES * CPC, SW], bf16)
            a2a_out = dram.tile([N_CORES * CPC, SW], bf16)

            # ---- phases 0-2, braided emission ------------------------
            # Engines execute their scheduled streams in static order, so
            # overlap must be built into emission order: the prep work
            # (x-load/transpose/qkv/V') for block n+1 is interleaved chunk-
            # by-chunk between the attention groups of q-block n.  Attention
            # qb=n depends only on qkv blocks 0..n, so each braid is legal.
            # PSUM banks: pA 2x1 + sT 2x2 + yt0 1 + yt1 1 = 8
            with (
                tc.tile_pool(name="xp", bufs=1) as xp,
                tc.tile_pool(name="xstg", bufs=3) as xstg,
                tc.tile_pool(name="ps", bufs=2, space="PSUM") as ps,
            ):
                xT = xp.tile([128, 8, seq], bf16)   # [c-chunk part, chunk, T]

                def wqkv_stage():
                    for k in range(8):
                        wtmp = xstg.tile([128, 3 * CPC], f32, tag="xf",
                                         bufs=3, name=f"wtmp_{k}")
                        nc.sync.dma_start(wtmp[:],
                                          wq_d[128 * k:128 * (k + 1), :])
                        nc.vector.tensor_copy(wqkv[:, k, :], wtmp[:])

                def prep_chunks(n):
                    """Emit-closures for block n: loads, x^T, qkv^T, V'."""
                    state = {}

                    def loads():
                        xbs = []
                        for u in range(4):
                            t = 4 * n + u
                            xf = xstg.tile([128, D_MODEL], f32, tag="xf",
                                           bufs=3, name=f"xf_{t}")
                            nc.sync.dma_start(xf[:],
                                              x_d[128 * t:128 * (t + 1), :])
                            xb = xstg.tile([128, D_MODEL], bf16, tag="xb",
                                           bufs=4, name=f"xb_{t}")
                            nc.gpsimd.tensor_copy(xb[:], xf[:])
                            xbs.append(xb)
                        state["xbs"] = xbs

                    def trans(j):
                        # j indexes (x-tile u = j//2, c-chunk quad a = j%2):
                        # one PSUM tile holds 4 c-chunk transposes of a
                        # single x-tile, so work starts after its one load
                        def emit():
                            u, a = divmod(j, 2)
                            tp = ps.tile([128, 512], f32, tag="pA",
                                         name=f"tp_{n}_{j}")
                            for c in range(4):
                                nc.tensor.matmul(
                                    tp[:, 128 * c:128 * (c + 1)],
                                    state["xbs"][u][:, 128 * (4 * a + c):
                                                    128 * (4 * a + c + 1)],
                                    iden[:], start=True, stop=True)
                            nc.vector.tensor_copy(
                                xT[:, 4 * a:4 * (a + 1),
                                   128 * (4 * n + u):128 * (4 * n + u + 1)],
                                tp[:])
                        return emit

                    def qkv(m):
                        def emit():
                            qp = ps.tile([128, 512], f32, tag="pA",
                                         name=f"qp_{n}_{m}")
                            for k in range(8):
                                nc.tensor.matmul(
                                    qp[:],
                                    wqkv[:, k, 128 * m:128 * (m + 1)],
                                    xT[:, k, 512 * n:512 * (n + 1)],
                                    start=(k == 0), stop=(k == 7))
                            if m == 0:
                                nc.vector.tensor_copy(
                                    qT[:, 512 * n:512 * (n + 1)], qp[:])
                            elif m == 1:
                                nc.vector.tensor_copy(
                                    kT[:, 512 * n:512 * (n + 1)], qp[:])
                            else:
                                vs = xstg.tile([128, 512], bf16, tag="vs",
                                               bufs=2, name=f"vs_{n}")
                                nc.vector.tensor_copy(vs[:], qp[:])
                                state["vs"] = vs
                        return emit

                    def vtr(u):
                        def emit():
                            t = 4 * n + u
                            vs = state["vs"]
                            # separate PSUM tiles per head: PE-write plus
                            # DVE-read of one PSUM bank is a HW fault
                            vp0 = ps.tile([128, 64], f32, tag="pA",
                                          name=f"vp0_{t}")
                            vp1 = ps.tile([128, 64], f32, tag="pA",
                                          name=f"vp1_{t}")
                            nc.tensor.matmul(
                                vp0[:], vs[0:64, 128 * u:128 * (u + 1)],
                                iden[0:64, 0:64], start=True, stop=True)
                            nc.tensor.matmul(
                                vp1[:], vs[64:128, 128 * u:128 * (u + 1)],
                                iden[64:128, 64:128], start=True, stop=True)
                            nc.vector.tensor_copy(Vp[:, t, 0:64], vp0[:])
                            nc.vector.tensor_copy(Vp[:, t, 65:129], vp1[:])
                        return emit

                    return ([loads] + [trans(j) for j in range(8)]
                            + [qkv(m) for m in range(3)]
                            + [vtr(u) for u in range(4)])

                def attention_groups(qb, ytps):
                    nkt = 4 * (qb + 1)

                    def group(g):
                        # diagonal k-tiles (d = kt-4qb >= 0) only attend to
                        # q >= 128d: trim score MM / exp / mask / yT MM to
                        # the valid column range [128d, QB).  q-cols below
                        # that are fully masked and, because kt=0 always
                        # covers the full width with start=True, never read.
                        def off(kt):
                            d = kt - 4 * qb
                            return 128 * d if d >= 0 else 0

                        def emit():
                            # h-inner MM order: consecutive score matmuls use
                            # disjoint PE row-groups (h0 rows 0-63, h1 rows
                            # 64-127) so the 16x32x32-subarray PE overlaps
                            # them (K=64 packing, ~2x on the score matmuls)
                            sps = [ps.tile([128, 2 * QB], f32, tag="sT",
                                           name=f"sp_{qb}_{g}_{h}")
                                   for h in (0, 1)]
                            for u in (0, 1):
                                kt = 2 * g + u
                                o = off(kt)
                                for h in (0, 1):
                                    nc.tensor.matmul(
                                        sps[h][:, QB * u + o:QB * (u + 1)],
                                        kT[64 * h:64 * (h + 1),
                                           128 * kt:128 * (kt + 1)],
                                        qT[64 * h:64 * (h + 1),
                                           QB * qb + o:QB * (qb + 1)],
                                        start=True, stop=True)
                            diag = off(2 * g) > 0 or off(2 * g + 1) > 0
                            for h in (0, 1):
                                pt = stg.tile([128, 2 * QB], bf16, tag="pT",
                                              bufs=3, name=f"pt_{qb}_{g}_{h}")
                                if diag:
                                    for u in (0, 1):
                                        o = off(2 * g + u)
                                        nc.scalar.activation(
                                            pt[:, QB * u + o:QB * (u + 1)],
                                            sps[h][:, QB * u + o:QB * (u + 1)],
                                            AFT.Exp, scale=0.125)
                                else:
                                    nc.scalar.activation(pt[:], sps[h][:],
                                                         AFT.Exp, scale=0.125)
                                for u in (0, 1):
                                    kt = 2 * g + u
                                    d = kt - 4 * qb
                                    o = off(kt)
                                    if d >= 0:
                                        w = min(o + 128, QB)
                                        nc.vector.tensor_mul(
                                            pt[:, QB * u + o:QB * u + w],
                                            pt[:, QB * u + o:QB * u + w],
                                            mks[:, d, o:w])
                                    nc.tensor.matmul(
                                        ytps[h][:, o:QB],
                                        Vp[:, kt, 65 * h:65 * (h + 1)],
                                        pt[:, QB * u + o:QB * (u + 1)],
                                        start=(kt == 0),
                                        stop=(kt == nkt - 1))
                        return emit

                    return [group(g) for g in range(nkt // 2)]

                def normalize(qb, ytps):
                    for h in (0, 1):
                        # one copy frees the PSUM accumulator right away; the
                        # denom -> 1/denom -> broadcast -> scale chain then
                        # runs from SBUF off the critical path.
                        yu = stg.tile([65, 2 * QB], f32, tag="dn", bufs=4,
                                      name=f"yu_{qb}_{h}")
                        nc.vector.tensor_copy(yu[:, 0:QB], ytps[h][:])
                        nc.scalar.activation(yu[64:65, QB:2 * QB],
                                             yu[64:65, 0:QB], AFT.Ln)
                        nc.scalar.activation(yu[64:65, QB:2 * QB],
                                             yu[64:65, QB:2 * QB], AFT.Exp,
                                             scale=-1.0)
                        bcp = ps.tile([64, QB], f32, tag="pA",
                                      name=f"bcp_{qb}_{h}")
                        nc.tensor.matmul(bcp[:], ones[64:65, 0:64],
                                         yu[64:65, QB:2 * QB],
                                         start=True, stop=True)
                        yn = yn0 if h == 0 else yn1
                        nc.vector.tensor_mul(yn[:, QB * qb:QB * (qb + 1)],
                                             yu[0:64, 0:QB], bcp[:])
                        if SW == QB:
                            # q-block == shard: stage its AllToAll rows now
                            j = qb
                            r0 = 128 * j + 64 * h
                            nc.sync.dma_start(a2a_in[r0:r0 + 64, :],
                                              yn[:, SW * j:SW * (j + 1)])

                def wpj_chunk(k):
                    def emit():
                        # w_proj staged late (projection tail only) and
                        # braided into the final attention block, which has
                        # no other prep work to overlap with
                        ptmp = xstg.tile([128, D_MODEL], f32, tag="xf",
                                         bufs=3, name=f"ptmp_{k}")
                        nc.sync.dma_start(ptmp[:],
                                          wp_d[128 * k:128 * (k + 1), :])
                        nc.vector.tensor_copy(wpj[:, k, :], ptmp[:])
                    return emit

                p0 = prep_chunks(0)
                p0[0]()           # stage-0 x loads lead the DMA queues
                wqkv_stage()
                for c in p0[1:]:
                    c()
                for n in range(nQB):
                    ytps = [ps.tile([65, QB], f32, tag=f"yt{h}", bufs=1,
                                    name=f"yt{h}_{n}") for h in (0, 1)]
                    if n + 1 < nQB:
                        pend = prep_chunks(n + 1)
                    else:
                        pend = [wpj_chunk(k) for k in range(8)]
                    groups = attention_groups(n, ytps)
                    ci = 0
                    for gi, g in enumerate(groups):
                        g()
                        want = (gi + 1) * len(pend) // len(groups)
                        while ci < want:
                            pend[ci]()
                            ci += 1
                    while ci < len(pend):
                        pend[ci]()
                        ci += 1
                    normalize(n, ytps)

            # ---- phase 3: AllToAll head-shard -> seq-shard ----------------
            if SW != QB:
                for j in range(N_CORES):
                    nc.sync.dma_start(a2a_in[128 * j:128 * j + 64, :],
                                      yn0[:, SW * j:SW * (j + 1)])
                    nc.sync.dma_start(a2a_in[128 * j + 64:128 * (j + 1), :],
                                      yn1[:, SW * j:SW * (j + 1)])
            if use_collective:
                nc.gpsimd.collective_compute(
                    "AllToAll", mybir.AluOpType.bypass,
                    ins=[a2a_in.opt()], outs=[a2a_out.opt()],
                    replica_groups=[list(range(N_CORES))])
            else:
                # timing-model variant (TimelineSim can't simulate
                # collectives): stand-in DRAM->DRAM copy
                nc.sync.dma_start(a2a_out[:], a2a_in[:])
            for j in range(N_CORES):
                nc.sync.dma_start(a2a_sb[:, j, :],
                                  a2a_out[128 * j:128 * (j + 1), :])

            # ---- phase 4: output projection for this core's SW rows -------
            with tc.tile_pool(name="psC", bufs=2, space="PSUM") as psC:
                mw = min(128, SW)
                for m in range(SW // mw):
                    pp = psC.tile([mw, D_MODEL], f32, tag="pp")
                    for n2 in (0, 1):
                        for k in range(8):
                            nc.tensor.matmul(
                                pp[:, 512 * n2:512 * (n2 + 1)],
                                a2a_sb[:, k, mw * m:mw * (m + 1)],
                                wpj[:, k, 512 * n2:512 * (n2 + 1)],
                                start=(k == 0), stop=(k == 7))
                    ob = stg.tile([mw, D_MODEL], f32, tag="ob", bufs=2)
                    nc.vector.tensor_copy(ob[:], pp[:])
                    nc.sync.dma_start(out_d[mw * m:mw * (m + 1), :], ob[:])

    if split_waits:
        _split_multi_waits(nc)
    return nc


def make_aux_inputs():
    ident = np.eye(128, dtype=BF16)
    k_idx = np.arange(128)[:, None]
    q_idx = np.arange(QB)[None, :]
    masks = np.stack(
        [((k_idx + 128 * d) <= q_idx).astype(BF16) for d in range(4)], axis=0)
    return ident, masks


def make_in_maps(x, w_qkv, w_proj, seq=SEQ):
    x = np.asarray(x, dtype=np.float32).reshape(seq, D_MODEL)
    w_qkv = np.asarray(w_qkv, dtype=np.float32)
    w_proj = np.asarray(w_proj, dtype=np.float32)
    ident, masks = make_aux_inputs()
    in_maps = []
    for i in range(N_CORES):
        sl = slice(CPC * i, CPC * (i + 1))
        w_slice = np.concatenate(
            [w_qkv[:, sl], w_qkv[:, D_MODEL:][:, sl],
             w_qkv[:, 2 * D_MODEL:][:, sl]], axis=1)
        in_maps.append({
            "x": x,
            "w_slice": np.ascontiguousarray(w_slice),
            "w_proj": w_proj,
            "ident": ident,
            "masks": masks,
        })
    return in_maps


_NC_CACHE = {}


def kernel(x, w_qkv, w_proj):
    """Full inputs in, full output out. Shards internally across 8 cores."""
    try:
        import os
        import jax
        jax.config.update("jax_compilation_cache_dir",
                          os.path.expanduser("~/.cache/jax_bass_kernel"))
        jax.config.update("jax_persistent_cache_min_compile_time_secs", 0.0)
    except Exception:
        pass
    from concourse.bass_utils import run_bass_kernel_spmd

    x = np.asarray(x, dtype=np.float32)
    batch = x.shape[0]
    seq = x.shape[1]
    if seq not in _NC_CACHE:
        _NC_CACHE[seq] = build_nc(seq)
    nc = _NC_CACHE[seq]
    in_maps = make_in_maps(x, w_qkv, w_proj, seq=seq)
    res = run_bass_kernel_spmd(nc, in_maps, list(range(N_CORES)))
    out = np.concatenate([res.results[j]["out"] for j in range(N_CORES)],
                         axis=0)
    return out.reshape(batch, seq, D_MODEL).astype(np.float32)



# revision 24
# speedup vs baseline: 1.0118x; 1.0015x over previous
"""Causal self-attention (d_model=1024, n_head=16, seq=4096) on 8 trn2 cores.

Sharding: tensor-parallel over heads (2 heads/core) for QKV + attention,
then an AllToAll re-shards y^T from head-sharded to sequence-sharded, so
each core runs the output projection for seq/8 rows with the full w_proj
(no AllReduce). The host concatenates the 8 row-shards.

Per-core layout (bf16 into the PE, fp32 PSUM accumulation):
  - x^T built via PE identity-matmul transposes (the d_model contraction
    needs x in [c, T] layout for both qkv operands).
  - qkv^T = w_slice.T @ x^T lands directly in [chan, T] layout, so qT/kT
    are exactly the lhsT/rhs of the score matmul (scores^T = K Q^T), and
    V' (normal orientation + a ones column) comes from small PE transposes.
  - softmax without max-subtraction (scores ~ N(0,1): exp cannot overflow
    fp32); the denominator falls out of the y^T matmul as the ones-column
    row; normalization uses exp(-ln(denom)) on ScalarE plus a K=1 matmul
    to broadcast the reciprocal across partitions.
  - causal masking: only lower-triangle k-tiles are computed; diagonal
    tiles are masked by a precomputed 0/1 multiply after the exp.
  - emission is braided: prep for block n+1 (x load/transpose/qkv/V') is
    interleaved between the attention groups of q-block n, under a single
    shared PSUM pool, so PE/ACT/DVE/DMA overlap across phases.
"""

import sys
import types

import numpy as np
import ml_dtypes

D_MODEL = 1024
N_HEAD = 16
SEQ = 4096
N_CORES = 8
D_HEAD = 64
CPC = 128            # channels per core (2 heads x 64)
QB = 512             # attention q-block width
BF16 = ml_dtypes.bfloat16


def _install_compat_patches():
    """Stub antenv.axon_hooks (absent in this container) so
    run_bass_kernel_spmd's trace path degrades instead of ImportError."""
    if "antenv.axon_hooks" not in sys.modules:
        mod = types.ModuleType("antenv.axon_hooks")
        mod.get_axon_ntff_profile_hook = lambda: None
        sys.modules["antenv.axon_hooks"] = mod


def _split_multi_waits(nc):
    """The nix walrus here accepts at most ONE sync-wait per instruction
    (setupSyncWait: 'Too many sync wait commands').  Hoist extra waits onto
    same-engine NoOps inserted immediately before the instruction — engine
    streams execute in program order, so semantics are unchanged."""
    import concourse.mybir as mybir

    n = 0
    for fn in nc.m.functions:
        for bb in fn.blocks:
            insts = bb.instructions
            out = []
            for inst in insts:
                si = getattr(inst, "sync_info", None)
                waits = list(si.on_wait) if si is not None else []
                if len(waits) > 1:
                    si.on_wait.clear()
                    for w in waits[:-1]:
                        n += 1
                        nop = mybir.InstNoOp(name=f"I-WSPLIT{n}", ins=[], outs=[])
                        nop.engine = inst.engine
                        nop.sync_info = mybir.SyncInfo(on_wait=[w], on_update=[])
                        out.append(nop)
                    si.on_wait.append(waits[-1])
                out.append(inst)
            bb.instructions = out


def build_nc(seq=SEQ, use_collective=True, split_waits=True):
    """Build the single-core SPMD program (identical on all 8 cores)."""
    import concourse.bass as bass
    import concourse.mybir as mybir
    from concourse.tile import TileContext

    _install_compat_patches()

    f32 = mybir.dt.float32
    bf16 = mybir.dt.bfloat16
    AFT = mybir.ActivationFunctionType

    nT = seq // 128       # T-tiles
    nQB = seq // QB       # attention q-blocks
    SW = seq // N_CORES   # AllToAll shard width (output rows per core)

    nc = bass.Bass("TRN2", target_bir_lowering=False, debug=False,
                   num_devices=N_CORES)
    x_d = nc.dram_tensor("x", [seq, D_MODEL], f32, kind="ExternalInput").ap()
    wq_d = nc.dram_tensor("w_slice", [D_MODEL, 3 * CPC], f32,
                          kind="ExternalInput").ap()
    wp_d = nc.dram_tensor("w_proj", [D_MODEL, D_MODEL], f32,
                          kind="ExternalInput").ap()
    id_d = nc.dram_tensor("ident", [128, 128], bf16, kind="ExternalInput").ap()
    mk_d = nc.dram_tensor("masks", [4, 128, QB], bf16,
                          kind="ExternalInput").ap()
    out_d = nc.dram_tensor("out", [SW, D_MODEL], f32,
                           kind="ExternalOutput").ap()

    with TileContext(nc) as tc:
        with (
            tc.tile_pool(name="per", bufs=1) as per,
            tc.tile_pool(name="stg", bufs=2) as stg,
            tc.tile_pool(name="dram", bufs=1, space="DRAM") as dram,
        ):
            qT = per.tile([128, seq], bf16)      # [2 heads x 64 d, T]
            kT = per.tile([128, seq], bf16)
            Vp = per.tile([128, nT, 130], bf16)  # V' tiles: [v_h0|1|v_h1|1]
            yn0 = per.tile([64, seq], bf16)      # normalized y^T, head 0
            yn1 = per.tile([64, seq], bf16)
            wqkv = per.tile([128, 8, 3 * CPC], bf16)
            wpj = per.tile([128, 8, D_MODEL], bf16)
            iden = per.tile([128, 128], bf16)
            mks = per.tile([128, 4, QB], bf16)
            ones = per.tile([128, 64], f32)
            a2a_sb = per.tile([128, 8, SW], bf16)

            nc.sync.dma_start(iden[:], id_d[:])
            for m in range(4):
                nc.sync.dma_start(mks[:, m, :], mk_d[m])
            nc.any.memset(ones[:], 1.0)
            nc.any.memset(Vp[:, :, 64:65], 1.0)
            nc.any.memset(Vp[:, :, 129:130], 1.0)

            # (weight staging happens inside the xstg pool below)

            a2a_in = dram.tile([N_CORES * CPC, SW], bf16)
            a2a_out = dram.tile([N_CORES * CPC, SW], bf16)

            # ---- phases 0-2, braided emission ------------------------
            # Engines execute their scheduled streams in static order, so
            # overlap must be built into emission order: the prep work
            # (x-load/transpose/qkv/V') for block n+1 is interleaved chunk-
            # by-chunk between the attention groups of q-block n.  Attention
            # qb=n depends only on qkv blocks 0..n, so each braid is legal.
            # PSUM banks: pA 2x1 + sT 2x2 + yt0 1 + yt1 1 = 8
            with (
                tc.tile_pool(name="xp", bufs=1) as xp,
                tc.tile_pool(name="xstg", bufs=3) as xstg,
                tc.tile_pool(name="ps", bufs=2, space="PSUM") as ps,
            ):
                xT = xp.tile([128, 8, seq], bf16)   # [c-chunk part, chunk, T]

                def wqkv_stage():
                    for k in range(8):
                        wtmp = xstg.tile([128, 3 * CPC], f32, tag="xf",
                                         bufs=3, name=f"wtmp_{k}")
                        nc.sync.dma_start(wtmp[:],
                                          wq_d[128 * k:128 * (k + 1), :])
                        nc.vector.tensor_copy(wqkv[:, k, :], wtmp[:])

                def prep_chunks(n):
                    """Emit-closures for block n: loads, x^T, qkv^T, V'."""
                    state = {}

                    def loads():
                        xbs = []
                        for u in range(4):
                            t = 4 * n + u
                            xf = xstg.tile([128, D_MODEL], f32, tag="xf",
                                           bufs=3, name=f"xf_{t}")
                            nc.sync.dma_start(xf[:],
                                              x_d[128 * t:128 * (t + 1), :])
                            xb = xstg.tile([128, D_MODEL], bf16, tag="xb",
                                           bufs=4, name=f"xb_{t}")
                            nc.gpsimd.tensor_copy(xb[:], xf[:])
                            xbs.append(xb)
                        state["xbs"] = xbs

                    def trans(j):
                        # j indexes (x-tile u = j//2, c-chunk quad a = j%2):
                        # one PSUM tile holds 4 c-chunk transposes of a
                        # single x-tile, so work starts after its one load
                        def emit():
                            u, a = divmod(j, 2)
                            tp = ps.tile([128, 512], f32, tag="pA",
                                         name=f"tp_{n}_{j}")
                            for c in range(4):
                                nc.tensor.matmul(
                                    tp[:, 128 * c:128 * (c + 1)],
                                    state["xbs"][u][:, 128 * (4 * a + c):
                                                    128 * (4 * a + c + 1)],
                                    iden[:], start=True, stop=True)
                            nc.vector.tensor_copy(
                                xT[:, 4 * a:4 * (a + 1),
                                   128 * (4 * n + u):128 * (4 * n + u + 1)],
                                tp[:])
                        return emit

                    def qkv(m):
                        def emit():
                            qp = ps.tile([128, 512], f32, tag="pA",
                                         name=f"qp_{n}_{m}")
                            for k in range(8):
                                nc.tensor.matmul(
                                    qp[:],
                                    wqkv[:, k, 128 * m:128 * (m + 1)],
                                    xT[:, k, 512 * n:512 * (n + 1)],
                                    start=(k == 0), stop=(k == 7))
                            if m == 0:
                                nc.vector.tensor_copy(
                                    qT[:, 512 * n:512 * (n + 1)], qp[:])
                            elif m == 1:
                                nc.vector.tensor_copy(
                                    kT[:, 512 * n:512 * (n + 1)], qp[:])
                            else:
                                vs = xstg.tile([128, 512], bf16, tag="vs",
                                               bufs=2, name=f"vs_{n}")
                                nc.vector.tensor_copy(vs[:], qp[:])
                                state["vs"] = vs
                        return emit

                    def vtr(u):
                        def emit():
                            t = 4 * n + u
                            vs = state["vs"]
                            # separate PSUM tiles per head: PE-write plus
                            # DVE-read of one PSUM bank is a HW fault
                            vp0 = ps.tile([128, 64], f32, tag="pA",
                                          name=f"vp0_{t}")
                            vp1 = ps.tile([128, 64], f32, tag="pA",
                                          name=f"vp1_{t}")
                            nc.tensor.matmul(
                                vp0[:], vs[0:64, 128 * u:128 * (u + 1)],
                                iden[0:64, 0:64], start=True, stop=True)
                            nc.tensor.matmul(
                                vp1[:], vs[64:128, 128 * u:128 * (u + 1)],
                                iden[64:128, 64:128], start=True, stop=True)
                            nc.vector.tensor_copy(Vp[:, t, 0:64], vp0[:])
                            nc.vector.tensor_copy(Vp[:, t, 65:129], vp1[:])
                        return emit

                    return ([loads] + [trans(j) for j in range(8)]
                            + [qkv(m) for m in range(3)]
                            + [vtr(u) for u in range(4)])

                def attention_groups(qb, ytps):
                    nkt = 4 * (qb + 1)

                    def group(g):
                        # diagonal k-tiles (d = kt-4qb >= 0) only attend to
                        # q >= 128d: trim score MM / exp / mask / yT MM to
                        # the valid column range [128d, QB).  q-cols below
                        # that are fully masked and, because kt=0 always
                        # covers the full width with start=True, never read.
                        def off(kt):
                            d = kt - 4 * qb
                            return 128 * d if d >= 0 else 0

                        def emit():
                            # h-inner MM order: consecutive score matmuls use
                            # disjoint PE row-groups (h0 rows 0-63, h1 rows
                            # 64-127) so the 16x32x32-subarray PE overlaps
                            # them (K=64 packing, ~2x on the score matmuls)
                            sps = [ps.tile([128, 2 * QB], f32, tag="sT",
                                           name=f"sp_{qb}_{g}_{h}")
                                   for h in (0, 1)]
                            for u in (0, 1):
                                kt = 2 * g + u
                                o = off(kt)
                                for h in (0, 1):
                                    nc.tensor.matmul(
                                        sps[h][:, QB * u + o:QB * (u + 1)],
                                        kT[64 * h:64 * (h + 1),
                                           128 * kt:128 * (kt + 1)],
                                        qT[64 * h:64 * (h + 1),
                                           QB * qb + o:QB * (qb + 1)],
                                        start=True, stop=True)
                            diag = off(2 * g) > 0 or off(2 * g + 1) > 0
                            for h in (0, 1):
                                pt = stg.tile([128, 2 * QB], bf16, tag="pT",
                                              bufs=3, name=f"pt_{qb}_{g}_{h}")
                                if diag:
                                    for u in (0, 1):
                                        o = off(2 * g + u)
                                        nc.scalar.activation(
                                            pt[:, QB * u + o:QB * (u + 1)],
                                            sps[h][:, QB * u + o:QB * (u + 1)],
                                            AFT.Exp, scale=0.125)
                                else:
                                    nc.scalar.activation(pt[:], sps[h][:],
                                                         AFT.Exp, scale=0.125)
                                for u in (0, 1):
                                    kt = 2 * g + u
                                    d = kt - 4 * qb
                                    o = off(kt)
                                    if d >= 0:
                                        w = min(o + 128, QB)
                                        nc.vector.tensor_mul(
                                            pt[:, QB * u + o:QB * u + w],
                                            pt[:, QB * u + o:QB * u + w],
                                            mks[:, d, o:w])
                                    nc.tensor.matmul(
                                        ytps[h][:, o:QB],
                                        Vp[:, kt, 65 * h:65 * (h + 1)],
                                        pt[:, QB * u + o:QB * (u + 1)],
                                        start=(kt == 0),
                                        stop=(kt == nkt - 1))
                        return emit

                    return [group(g) for g in range(nkt // 2)]

                def normalize(qb, ytps):
                    for h in (0, 1):
                        # one copy frees the PSUM accumulator right away; the
                        # denom -> 1/denom -> broadcast -> scale chain then
                        # runs from SBUF off the critical path.
                        yu = stg.tile([65, 2 * QB], f32, tag="dn", bufs=4,
                                      name=f"yu_{qb}_{h}")
                        nc.vector.tensor_copy(yu[:, 0:QB], ytps[h][:])
                        nc.scalar.activation(yu[64:65, QB:2 * QB],
                                             yu[64:65, 0:QB], AFT.Ln)
                        nc.scalar.activation(yu[64:65, QB:2 * QB],
                                             yu[64:65, QB:2 * QB], AFT.Exp,
                                             scale=-1.0)
                        bcp = ps.tile([64, QB], f32, tag="pA",
                                      name=f"bcp_{qb}_{h}")
                        nc.tensor.matmul(bcp[:], ones[64:65, 0:64],
                                         yu[64:65, QB:2 * QB],
                                         start=True, stop=True)
                        yn = yn0 if h == 0 else yn1
                        nc.vector.tensor_mul(yn[:, QB * qb:QB * (qb + 1)],
                                             yu[0:64, 0:QB], bcp[:])
                        if SW == QB:
                            # q-block == shard: stage its AllToAll rows now
                            j = qb
                            r0 = 128 * j + 64 * h
                            nc.sync.dma_start(a2a_in[r0:r0 + 64, :],
                                              yn[:, SW * j:SW * (j + 1)])

                def wpj_chunk(k):
                    def emit():
                        # w_proj staged late (projection tail only) and
                        # braided into the final attention block, which has
                        # no other prep work to overlap with
                        ptmp = xstg.tile([128, D_MODEL], f32, tag="xf",
                                         bufs=3, name=f"ptmp_{k}")
                        nc.sync.dma_start(ptmp[:],
                                          wp_d[128 * k:128 * (k + 1), :])
                        nc.gpsimd.tensor_copy(wpj[:, k, :], ptmp[:])
                    return emit

                p0 = prep_chunks(0)
                p0[0]()           # stage-0 x loads lead the DMA queues
                wqkv_stage()
                for c in p0[1:]:
                    c()
                for n in range(nQB):
                    ytps = [ps.tile([65, QB], f32, tag=f"yt{h}", bufs=1,
                                    name=f"yt{h}_{n}") for h in (0, 1)]
                    if n + 1 < nQB:
                        pend = prep_chunks(n + 1)
                    else:
                        pend = [wpj_chunk(k) for k in range(8)]
                    groups = attention_groups(n, ytps)
                    ci = 0
                    for gi, g in enumerate(groups):
                        g()
                        want = (gi + 1) * len(pend) // len(groups)
                        while ci < want:
                            pend[ci]()
                            ci += 1
                    while ci < len(pend):
                        pend[ci]()
                        ci += 1
                    normalize(n, ytps)

            # ---- phase 3: AllToAll head-shard -> seq-shard ----------------
            if SW != QB:
                for j in range(N_CORES):
                    nc.sync.dma_start(a2a_in[128 * j:128 * j + 64, :],
                                      yn0[:, SW * j:SW * (j + 1)])
                    nc.sync.dma_start(a2a_in[128 * j + 64:128 * (j + 1), :],
                                      yn1[:, SW * j:SW * (j + 1)])
            if use_collective:
                nc.gpsimd.collective_compute(
                    "AllToAll", mybir.AluOpType.bypass,
                    ins=[a2a_in.opt()], outs=[a2a_out.opt()],
                    replica_groups=[list(range(N_CORES))])
            else:
                # timing-model variant (TimelineSim can't simulate
                # collectives): stand-in DRAM->DRAM copy
                nc.sync.dma_start(a2a_out[:], a2a_in[:])
            for j in range(N_CORES):
                nc.sync.dma_start(a2a_sb[:, j, :],
                                  a2a_out[128 * j:128 * (j + 1), :])

            # ---- phase 4: output projection for this core's SW rows -------
            with tc.tile_pool(name="psC", bufs=2, space="PSUM") as psC:
                mw = min(128, SW)
                for m in range(SW // mw):
                    pp = psC.tile([mw, D_MODEL], f32, tag="pp")
                    for n2 in (0, 1):
                        for k in range(8):
                            nc.tensor.matmul(
                                pp[:, 512 * n2:512 * (n2 + 1)],
                                a2a_sb[:, k, mw * m:mw * (m + 1)],
                                wpj[:, k, 512 * n2:512 * (n2 + 1)],
                                start=(k == 0), stop=(k == 7))
                    ob = stg.tile([mw, D_MODEL], f32, tag="ob", bufs=2)
                    nc.vector.tensor_copy(ob[:], pp[:])
                    nc.sync.dma_start(out_d[mw * m:mw * (m + 1), :], ob[:])

    if split_waits:
        _split_multi_waits(nc)
    return nc


def make_aux_inputs():
    ident = np.eye(128, dtype=BF16)
    k_idx = np.arange(128)[:, None]
    q_idx = np.arange(QB)[None, :]
    masks = np.stack(
        [((k_idx + 128 * d) <= q_idx).astype(BF16) for d in range(4)], axis=0)
    return ident, masks


def make_in_maps(x, w_qkv, w_proj, seq=SEQ):
    x = np.asarray(x, dtype=np.float32).reshape(seq, D_MODEL)
    w_qkv = np.asarray(w_qkv, dtype=np.float32)
    w_proj = np.asarray(w_proj, dtype=np.float32)
    ident, masks = make_aux_inputs()
    in_maps = []
    for i in range(N_CORES):
        sl = slice(CPC * i, CPC * (i + 1))
        w_slice = np.concatenate(
            [w_qkv[:, sl], w_qkv[:, D_MODEL:][:, sl],
             w_qkv[:, 2 * D_MODEL:][:, sl]], axis=1)
        in_maps.append({
            "x": x,
            "w_slice": np.ascontiguousarray(w_slice),
            "w_proj": w_proj,
            "ident": ident,
            "masks": masks,
        })
    return in_maps


_NC_CACHE = {}


def kernel(x, w_qkv, w_proj):
    """Full inputs in, full output out. Shards internally across 8 cores."""
    try:
        import os
        import jax
        jax.config.update("jax_compilation_cache_dir",
                          os.path.expanduser("~/.cache/jax_bass_kernel"))
        jax.config.update("jax_persistent_cache_min_compile_time_secs", 0.0)
    except Exception:
        pass
    from concourse.bass_utils import run_bass_kernel_spmd

    x = np.asarray(x, dtype=np.float32)
    batch = x.shape[0]
    seq = x.shape[1]
    if seq not in _NC_CACHE:
        _NC_CACHE[seq] = build_nc(seq)
    nc = _NC_CACHE[seq]
    in_maps = make_in_maps(x, w_qkv, w_proj, seq=seq)
    res = run_bass_kernel_spmd(nc, in_maps, list(range(N_CORES)))
    out = np.concatenate([res.results[j]["out"] for j in range(N_CORES)],
                         axis=0)
    return out.reshape(batch, seq, D_MODEL).astype(np.float32)



# revision 25
# speedup vs baseline: 1.0298x; 1.0178x over previous
"""Causal self-attention (d_model=1024, n_head=16, seq=4096) on 8 trn2 cores.

Sharding: tensor-parallel over heads (2 heads/core) for QKV + attention,
then an AllToAll re-shards y^T from head-sharded to sequence-sharded, so
each core runs the output projection for seq/8 rows with the full w_proj
(no AllReduce). The host concatenates the 8 row-shards.

Per-core layout (bf16 into the PE, fp32 PSUM accumulation):
  - x^T built via PE identity-matmul transposes (the d_model contraction
    needs x in [c, T] layout for both qkv operands).
  - qkv^T = w_slice.T @ x^T lands directly in [chan, T] layout, so qT/kT
    are exactly the lhsT/rhs of the score matmul (scores^T = K Q^T), and
    V' (normal orientation + a ones column) comes from small PE transposes.
  - softmax without max-subtraction (scores ~ N(0,1): exp cannot overflow
    fp32); the denominator falls out of the y^T matmul as the ones-column
    row; normalization uses exp(-ln(denom)) on ScalarE plus a K=1 matmul
    to broadcast the reciprocal across partitions.
  - causal masking: only lower-triangle k-tiles are computed; diagonal
    tiles are masked by a precomputed 0/1 multiply after the exp.
  - emission is braided: prep for block n+1 (x load/transpose/qkv/V') is
    interleaved between the attention groups of q-block n, under a single
    shared PSUM pool, so PE/ACT/DVE/DMA overlap across phases.
"""

import sys
import types

import numpy as np
import ml_dtypes

D_MODEL = 1024
N_HEAD = 16
SEQ = 4096
N_CORES = 8
D_HEAD = 64
CPC = 128            # channels per core (2 heads x 64)
QB = 512             # attention q-block width
BF16 = ml_dtypes.bfloat16
XBAR_FROM_BLOCK = 99   # blocks >= this: x^T via ACT-queue xbar into scratch
PBC_NORM = True       # normalize broadcast via gpsimd partition_broadcast


def _install_compat_patches():
    """Stub antenv.axon_hooks (absent in this container) so
    run_bass_kernel_spmd's trace path degrades instead of ImportError."""
    if "antenv.axon_hooks" not in sys.modules:
        mod = types.ModuleType("antenv.axon_hooks")
        mod.get_axon_ntff_profile_hook = lambda: None
        sys.modules["antenv.axon_hooks"] = mod


def _split_multi_waits(nc):
    """The nix walrus here accepts at most ONE sync-wait per instruction
    (setupSyncWait: 'Too many sync wait commands').  Hoist extra waits onto
    same-engine NoOps inserted immediately before the instruction — engine
    streams execute in program order, so semantics are unchanged."""
    import concourse.mybir as mybir

    n = 0
    for fn in nc.m.functions:
        for bb in fn.blocks:
            insts = bb.instructions
            out = []
            for inst in insts:
                si = getattr(inst, "sync_info", None)
                waits = list(si.on_wait) if si is not None else []
                if len(waits) > 1:
                    si.on_wait.clear()
                    for w in waits[:-1]:
                        n += 1
                        nop = mybir.InstNoOp(name=f"I-WSPLIT{n}", ins=[], outs=[])
                        nop.engine = inst.engine
                        nop.sync_info = mybir.SyncInfo(on_wait=[w], on_update=[])
                        out.append(nop)
                    si.on_wait.append(waits[-1])
                out.append(inst)
            bb.instructions = out


def build_nc(seq=SEQ, use_collective=True, split_waits=True):
    """Build the single-core SPMD program (identical on all 8 cores)."""
    import concourse.bass as bass
    import concourse.mybir as mybir
    from concourse.tile import TileContext

    _install_compat_patches()

    f32 = mybir.dt.float32
    bf16 = mybir.dt.bfloat16
    AFT = mybir.ActivationFunctionType

    from concourse import library_config

    nT = seq // 128       # T-tiles
    nQB = seq // QB       # attention q-blocks
    SW = seq // N_CORES   # AllToAll shard width (output rows per core)

    nc = bass.Bass("TRN2", target_bir_lowering=False, debug=False,
                   num_devices=N_CORES)
    x_d = nc.dram_tensor("x", [seq, D_MODEL], f32, kind="ExternalInput").ap()
    wq_d = nc.dram_tensor("w_slice", [D_MODEL, 3 * CPC], f32,
                          kind="ExternalInput").ap()
    wp_d = nc.dram_tensor("w_proj", [D_MODEL, D_MODEL], f32,
                          kind="ExternalInput").ap()
    id_d = nc.dram_tensor("ident", [128, 128], bf16, kind="ExternalInput").ap()
    mk_d = nc.dram_tensor("masks", [4, 128, QB], bf16,
                          kind="ExternalInput").ap()
    out_d = nc.dram_tensor("out", [SW, D_MODEL], f32,
                           kind="ExternalOutput").ap()

    with TileContext(nc) as tc:
        with (
            tc.tile_pool(name="per", bufs=1) as per,
            tc.tile_pool(name="stg", bufs=2) as stg,
            tc.tile_pool(name="dram", bufs=1, space="DRAM") as dram,
        ):
            qT = per.tile([128, seq], bf16)      # [2 heads x 64 d, T]
            kT = per.tile([128, seq], bf16)
            Vp = per.tile([128, nT, 130], bf16)  # V' tiles: [v_h0|1|v_h1|1]
            yn0 = per.tile([64, seq], bf16)      # normalized y^T, head 0
            yn1 = per.tile([64, seq], bf16)
            wqkv = per.tile([128, 8, 3 * CPC], bf16)
            wpj = per.tile([128, 8, D_MODEL], bf16)
            iden = per.tile([128, 128], bf16)
            mks = per.tile([128, 4, QB], bf16)
            ones = per.tile([128, 64], f32)
            a2a_sb = per.tile([128, 8, SW], bf16)

            nc.sync.dma_start(iden[:], id_d[:])
            for m in range(4):
                nc.sync.dma_start(mks[:, m, :], mk_d[m])
            nc.any.memset(ones[:], 1.0)
            nc.any.memset(Vp[:, :, 64:65], 1.0)
            nc.any.memset(Vp[:, :, 129:130], 1.0)

            # (weight staging happens inside the xstg pool below)

            dnd = dram.tile([16, QB], mybir.dt.float32)
            a2a_in = dram.tile([N_CORES * CPC, SW], bf16)
            a2a_out = dram.tile([N_CORES * CPC, SW], bf16)

            # ---- phases 0-2, braided emission ------------------------
            # Engines execute their scheduled streams in static order, so
            # overlap must be built into emission order: the prep work
            # (x-load/transpose/qkv/V') for block n+1 is interleaved chunk-
            # by-chunk between the attention groups of q-block n.  Attention
            # qb=n depends only on qkv blocks 0..n, so each braid is legal.
            # PSUM banks: pA 2x1 + sT 2x2 + yt0 1 + yt1 1 = 8
            with (
                tc.tile_pool(name="xp", bufs=1) as xp,
                tc.tile_pool(name="xstg", bufs=3) as xstg,
                tc.tile_pool(name="ps", bufs=2, space="PSUM") as ps,
            ):
                xT = xp.tile([128, 8, seq], bf16)   # [c-chunk part, chunk, T]

                def wqkv_stage():
                    for k in range(8):
                        wtmp = xstg.tile([128, 3 * CPC], f32, tag="xf",
                                         bufs=3, name=f"wtmp_{k}")
                        nc.sync.dma_start(wtmp[:],
                                          wq_d[128 * k:128 * (k + 1), :])
                        nc.vector.tensor_copy(wqkv[:, k, :], wtmp[:])

                def prep_chunks(n):
                    """Emit-closures for block n: loads, x^T, qkv^T, V'."""
                    state = {}

                    def loads():
                        xbs = []
                        for u in range(4):
                            t = 4 * n + u
                            xf = xstg.tile([128, D_MODEL], f32, tag="xf",
                                           bufs=3, name=f"xf_{t}")
                            nc.sync.dma_start(xf[:],
                                              x_d[128 * t:128 * (t + 1), :])
                            xb = xstg.tile([128, D_MODEL], bf16, tag="xb",
                                           bufs=4, name=f"xb_{t}")
                            nc.gpsimd.tensor_copy(xb[:], xf[:])
                            xbs.append(xb)
                        state["xbs"] = xbs

                    def xtr(u):
                        def emit():
                            t = 4 * n + u
                            xc = xstg.tile([128, 8, 128], bf16, tag="xc",
                                           bufs=3, name=f"xc_{t}")
                            nc.scalar.dma_start_transpose(
                                xc[:], state["xbs"][u][:])
                            nc.vector.tensor_copy(
                                xT[:, :, 128 * t:128 * (t + 1)], xc[:])
                        return emit

                    def trans(j):
                        # j indexes (x-tile u = j//2, c-chunk quad a = j%2):
                        # one PSUM tile holds 4 c-chunk transposes of a
                        # single x-tile, so work starts after its one load
                        def emit():
                            u, a = divmod(j, 2)
                            tp = ps.tile([128, 512], f32, tag="pA",
                                         name=f"tp_{n}_{j}")
                            for c in range(4):
                                nc.tensor.matmul(
                                    tp[:, 128 * c:128 * (c + 1)],
                                    state["xbs"][u][:, 128 * (4 * a + c):
                                                    128 * (4 * a + c + 1)],
                                    iden[:], start=True, stop=True)
                            nc.vector.tensor_copy(
                                xT[:, 4 * a:4 * (a + 1),
                                   128 * (4 * n + u):128 * (4 * n + u + 1)],
                                tp[:])
                        return emit

                    def qkv(m):
                        def emit():
                            qp = ps.tile([128, 512], f32, tag="pA",
                                         name=f"qp_{n}_{m}")
                            for k in range(8):
                                nc.tensor.matmul(
                                    qp[:],
                                    wqkv[:, k, 128 * m:128 * (m + 1)],
                                    xT[:, k, 512 * n:512 * (n + 1)],
                                    start=(k == 0), stop=(k == 7))
                            if m == 0:
                                nc.vector.tensor_copy(
                                    qT[:, 512 * n:512 * (n + 1)], qp[:])
                            elif m == 1:
                                nc.vector.tensor_copy(
                                    kT[:, 512 * n:512 * (n + 1)], qp[:])
                            else:
                                vs = xstg.tile([128, 512], bf16, tag="vs",
                                               bufs=2, name=f"vs_{n}")
                                nc.vector.tensor_copy(vs[:], qp[:])
                                state["vs"] = vs
                        return emit

                    def vtr(u):
                        def emit():
                            t = 4 * n + u
                            vs = state["vs"]
                            # separate PSUM tiles per head: PE-write plus
                            # DVE-read of one PSUM bank is a HW fault
                            vp0 = ps.tile([128, 64], f32, tag="pA",
                                          name=f"vp0_{t}")
                            vp1 = ps.tile([128, 64], f32, tag="pA",
                                          name=f"vp1_{t}")
                            nc.tensor.matmul(
                                vp0[:], vs[0:64, 128 * u:128 * (u + 1)],
                                iden[0:64, 0:64], start=True, stop=True)
                            nc.tensor.matmul(
                                vp1[:], vs[64:128, 128 * u:128 * (u + 1)],
                                iden[64:128, 64:128], start=True, stop=True)
                            nc.vector.tensor_copy(Vp[:, t, 0:64], vp0[:])
                            nc.vector.tensor_copy(Vp[:, t, 65:129], vp1[:])
                        return emit

                    if n >= XBAR_FROM_BLOCK:
                        return ([loads] + [xtr(u) for u in range(4)]
                                + [qkv(m) for m in range(3)]
                                + [vtr(u) for u in range(4)])
                    return ([loads] + [trans(j) for j in range(8)]
                            + [qkv(m) for m in range(3)]
                            + [vtr(u) for u in range(4)])

                def attention_groups(qb, ytps):
                    nkt = 4 * (qb + 1)

                    def group(g):
                        # diagonal k-tiles (d = kt-4qb >= 0) only attend to
                        # q >= 128d: trim score MM / exp / mask / yT MM to
                        # the valid column range [128d, QB).  q-cols below
                        # that are fully masked and, because kt=0 always
                        # covers the full width with start=True, never read.
                        def off(kt):
                            d = kt - 4 * qb
                            return 128 * d if d >= 0 else 0

                        def emit():
                            # h-inner MM order: consecutive score matmuls use
                            # disjoint PE row-groups (h0 rows 0-63, h1 rows
                            # 64-127) so the 16x32x32-subarray PE overlaps
                            # them (K=64 packing, ~2x on the score matmuls)
                            sps = [ps.tile([128, 2 * QB], f32, tag="sT",
                                           name=f"sp_{qb}_{g}_{h}")
                                   for h in (0, 1)]
                            for u in (0, 1):
                                kt = 2 * g + u
                                o = off(kt)
                                for h in (0, 1):
                                    nc.tensor.matmul(
                                        sps[h][:, QB * u + o:QB * (u + 1)],
                                        kT[64 * h:64 * (h + 1),
                                           128 * kt:128 * (kt + 1)],
                                        qT[64 * h:64 * (h + 1),
                                           QB * qb + o:QB * (qb + 1)],
                                        start=True, stop=True)
                            diag = off(2 * g) > 0 or off(2 * g + 1) > 0
                            for h in (0, 1):
                                pt = stg.tile([128, 2 * QB], bf16, tag="pT",
                                              bufs=3, name=f"pt_{qb}_{g}_{h}")
                                if diag:
                                    for u in (0, 1):
                                        o = off(2 * g + u)
                                        nc.scalar.activation(
                                            pt[:, QB * u + o:QB * (u + 1)],
                                            sps[h][:, QB * u + o:QB * (u + 1)],
                                            AFT.Exp, scale=0.125)
                                else:
                                    nc.scalar.activation(pt[:], sps[h][:],
                                                         AFT.Exp, scale=0.125)
                                for u in (0, 1):
                                    kt = 2 * g + u
                                    d = kt - 4 * qb
                                    o = off(kt)
                                    if d >= 0:
                                        w = min(o + 128, QB)
                                        nc.vector.tensor_mul(
                                            pt[:, QB * u + o:QB * u + w],
                                            pt[:, QB * u + o:QB * u + w],
                                            mks[:, d, o:w])
                                    nc.tensor.matmul(
                                        ytps[h][:, o:QB],
                                        Vp[:, kt, 65 * h:65 * (h + 1)],
                                        pt[:, QB * u + o:QB * (u + 1)],
                                        start=(kt == 0),
                                        stop=(kt == nkt - 1))
                        return emit

                    return [group(g) for g in range(nkt // 2)]

                def normalize(qb, ytps):
                    for h in (0, 1):
                        # one copy frees the PSUM accumulator right away; the
                        # denom -> 1/denom -> broadcast -> scale chain then
                        # runs from SBUF off the critical path.
                        yu = stg.tile([65, 2 * QB], f32, tag="dn", bufs=4,
                                      name=f"yu_{qb}_{h}")
                        nc.vector.tensor_copy(yu[:, 0:QB], ytps[h][:])
                        nc.scalar.activation(yu[64:65, QB:2 * QB],
                                             yu[64:65, 0:QB], AFT.Ln)
                        nc.scalar.activation(yu[64:65, QB:2 * QB],
                                             yu[64:65, QB:2 * QB], AFT.Exp,
                                             scale=-1.0)
                        yn = yn0 if h == 0 else yn1
                        if PBC_NORM:
                            bcs = stg.tile([64, QB], f32, tag="bc2", bufs=2,
                                           name=f"bcs_{qb}_{h}")
                            dslot = dnd[2 * qb + h:2 * qb + h + 1, :]
                            nc.sync.dma_start(dslot, yu[64:65, QB:2 * QB])
                            nc.sync.dma_start(
                                bcs[:], dslot.broadcast_to([64, QB]))
                            nc.vector.tensor_mul(
                                yn[:, QB * qb:QB * (qb + 1)],
                                yu[0:64, 0:QB], bcs[:])
                        else:
                            bcp = ps.tile([64, QB], f32, tag="pA",
                                          name=f"bcp_{qb}_{h}")
                            nc.tensor.matmul(bcp[:], ones[64:65, 0:64],
                                             yu[64:65, QB:2 * QB],
                                             start=True, stop=True)
                            nc.vector.tensor_mul(
                                yn[:, QB * qb:QB * (qb + 1)],
                                yu[0:64, 0:QB], bcp[:])
                        if SW == QB:
                            # q-block == shard: stage its AllToAll rows now
                            j = qb
                            r0 = 128 * j + 64 * h
                            nc.sync.dma_start(a2a_in[r0:r0 + 64, :],
                                              yn[:, SW * j:SW * (j + 1)])

                def wpj_chunk(k):
                    def emit():
                        # w_proj staged late (projection tail only) and
                        # braided into the final attention block, which has
                        # no other prep work to overlap with
                        ptmp = xstg.tile([128, D_MODEL], f32, tag="xf",
                                         bufs=3, name=f"ptmp_{k}")
                        nc.sync.dma_start(ptmp[:],
                                          wp_d[128 * k:128 * (k + 1), :])
                        nc.gpsimd.tensor_copy(wpj[:, k, :], ptmp[:])
                    return emit

                p0 = prep_chunks(0)
                p0[0]()           # stage-0 x loads lead the DMA queues
                wqkv_stage()
                for c in p0[1:]:
                    c()
                for n in range(nQB):
                    ytps = [ps.tile([65, QB], f32, tag=f"yt{h}", bufs=1,
                                    name=f"yt{h}_{n}") for h in (0, 1)]
                    if n + 1 < nQB:
                        pend = prep_chunks(n + 1)
                    else:
                        pend = [wpj_chunk(k) for k in range(8)]
                    groups = attention_groups(n, ytps)
                    ci = 0
                    for gi, g in enumerate(groups):
                        g()
                        want = (gi + 1) * len(pend) // len(groups)
                        while ci < want:
                            pend[ci]()
                            ci += 1
                    while ci < len(pend):
                        pend[ci]()
                        ci += 1
                    normalize(n, ytps)

            # ---- phase 3: AllToAll head-shard -> seq-shard ----------------
            if SW != QB:
                for j in range(N_CORES):
                    nc.sync.dma_start(a2a_in[128 * j:128 * j + 64, :],
                                      yn0[:, SW * j:SW * (j + 1)])
                    nc.sync.dma_start(a2a_in[128 * j + 64:128 * (j + 1), :],
                                      yn1[:, SW * j:SW * (j + 1)])
            if use_collective:
                nc.gpsimd.collective_compute(
                    "AllToAll", mybir.AluOpType.bypass,
                    ins=[a2a_in.opt()], outs=[a2a_out.opt()],
                    replica_groups=[list(range(N_CORES))])
            else:
                # timing-model variant (TimelineSim can't simulate
                # collectives): stand-in DRAM->DRAM copy
                nc.sync.dma_start(a2a_out[:], a2a_in[:])
            for j in range(N_CORES):
                nc.sync.dma_start(a2a_sb[:, j, :],
                                  a2a_out[128 * j:128 * (j + 1), :])

            # ---- phase 4: output projection for this core's SW rows -------
            with tc.tile_pool(name="psC", bufs=2, space="PSUM") as psC:
                mw = min(128, SW)
                for m in range(SW // mw):
                    pp = psC.tile([mw, D_MODEL], f32, tag="pp")
                    for n2 in (0, 1):
                        for k in range(8):
                            nc.tensor.matmul(
                                pp[:, 512 * n2:512 * (n2 + 1)],
                                a2a_sb[:, k, mw * m:mw * (m + 1)],
                                wpj[:, k, 512 * n2:512 * (n2 + 1)],
                                start=(k == 0), stop=(k == 7))
                    ob = stg.tile([mw, D_MODEL], f32, tag="ob", bufs=2)
                    nc.vector.tensor_copy(ob[:], pp[:])
                    nc.sync.dma_start(out_d[mw * m:mw * (m + 1), :], ob[:])

    if split_waits:
        _split_multi_waits(nc)
    return nc


def make_aux_inputs():
    ident = np.eye(128, dtype=BF16)
    k_idx = np.arange(128)[:, None]
    q_idx = np.arange(QB)[None, :]
    masks = np.stack(
        [((k_idx + 128 * d) <= q_idx).astype(BF16) for d in range(4)], axis=0)
    return ident, masks


def make_in_maps(x, w_qkv, w_proj, seq=SEQ):
    x = np.asarray(x, dtype=np.float32).reshape(seq, D_MODEL)
    w_qkv = np.asarray(w_qkv, dtype=np.float32)
    w_proj = np.asarray(w_proj, dtype=np.float32)
    ident, masks = make_aux_inputs()
    in_maps = []
    for i in range(N_CORES):
        sl = slice(CPC * i, CPC * (i + 1))
        w_slice = np.concatenate(
            [w_qkv[:, sl], w_qkv[:, D_MODEL:][:, sl],
             w_qkv[:, 2 * D_MODEL:][:, sl]], axis=1)
        in_maps.append({
            "x": x,
            "w_slice": np.ascontiguousarray(w_slice),
            "w_proj": w_proj,
            "ident": ident,
            "masks": masks,
        })
    return in_maps


_NC_CACHE = {}


def kernel(x, w_qkv, w_proj):
    """Full inputs in, full output out. Shards internally across 8 cores."""
    try:
        import os
        import jax
        jax.config.update("jax_compilation_cache_dir",
                          os.path.expanduser("~/.cache/jax_bass_kernel"))
        jax.config.update("jax_persistent_cache_min_compile_time_secs", 0.0)
    except Exception:
        pass
    from concourse.bass_utils import run_bass_kernel_spmd

    x = np.asarray(x, dtype=np.float32)
    batch = x.shape[0]
    seq = x.shape[1]
    if seq not in _NC_CACHE:
        _NC_CACHE[seq] = build_nc(seq)
    nc = _NC_CACHE[seq]
    in_maps = make_in_maps(x, w_qkv, w_proj, seq=seq)
    res = run_bass_kernel_spmd(nc, in_maps, list(range(N_CORES)))
    out = np.concatenate([res.results[j]["out"] for j in range(N_CORES)],
                         axis=0)
    return out.reshape(batch, seq, D_MODEL).astype(np.float32)



# revision 26
# speedup vs baseline: 1.0422x; 1.0121x over previous
"""Causal self-attention (d_model=1024, n_head=16, seq=4096) on 8 trn2 cores.

Sharding: tensor-parallel over heads (2 heads/core) for QKV + attention,
then an AllToAll re-shards y^T from head-sharded to sequence-sharded, so
each core runs the output projection for seq/8 rows with the full w_proj
(no AllReduce). The host concatenates the 8 row-shards.

Per-core layout (bf16 into the PE, fp32 PSUM accumulation):
  - x^T built via PE identity-matmul transposes (the d_model contraction
    needs x in [c, T] layout for both qkv operands).
  - qkv^T = w_slice.T @ x^T lands directly in [chan, T] layout, so qT/kT
    are exactly the lhsT/rhs of the score matmul (scores^T = K Q^T), and
    V' (normal orientation + a ones column) comes from small PE transposes.
  - softmax without max-subtraction (scores ~ N(0,1): exp cannot overflow
    fp32); the denominator falls out of the y^T matmul as the ones-column
    row; normalization uses exp(-ln(denom)) on ScalarE plus a K=1 matmul
    to broadcast the reciprocal across partitions.
  - causal masking: only lower-triangle k-tiles are computed; diagonal
    tiles are masked by a precomputed 0/1 multiply after the exp.
  - emission is braided: prep for block n+1 (x load/transpose/qkv/V') is
    interleaved between the attention groups of q-block n, under a single
    shared PSUM pool, so PE/ACT/DVE/DMA overlap across phases.
"""

import sys
import types

import numpy as np
import ml_dtypes

D_MODEL = 1024
N_HEAD = 16
SEQ = 4096
N_CORES = 8
D_HEAD = 64
CPC = 128            # channels per core (2 heads x 64)
QB = 512             # attention q-block width
BF16 = ml_dtypes.bfloat16
XBAR_FROM_BLOCK = 99   # blocks >= this: x^T via ACT-queue xbar into scratch
PBC_NORM = True       # normalize broadcast via gpsimd partition_broadcast


def _install_compat_patches():
    """Stub antenv.axon_hooks (absent in this container) so
    run_bass_kernel_spmd's trace path degrades instead of ImportError."""
    if "antenv.axon_hooks" not in sys.modules:
        mod = types.ModuleType("antenv.axon_hooks")
        mod.get_axon_ntff_profile_hook = lambda: None
        sys.modules["antenv.axon_hooks"] = mod


def _split_multi_waits(nc):
    """The nix walrus here accepts at most ONE sync-wait per instruction
    (setupSyncWait: 'Too many sync wait commands').  Hoist extra waits onto
    same-engine NoOps inserted immediately before the instruction — engine
    streams execute in program order, so semantics are unchanged."""
    import concourse.mybir as mybir

    n = 0
    for fn in nc.m.functions:
        for bb in fn.blocks:
            insts = bb.instructions
            out = []
            for inst in insts:
                si = getattr(inst, "sync_info", None)
                waits = list(si.on_wait) if si is not None else []
                if len(waits) > 1:
                    si.on_wait.clear()
                    for w in waits[:-1]:
                        n += 1
                        nop = mybir.InstNoOp(name=f"I-WSPLIT{n}", ins=[], outs=[])
                        nop.engine = inst.engine
                        nop.sync_info = mybir.SyncInfo(on_wait=[w], on_update=[])
                        out.append(nop)
                    si.on_wait.append(waits[-1])
                out.append(inst)
            bb.instructions = out


def build_nc(seq=SEQ, use_collective=True, split_waits=True):
    """Build the single-core SPMD program (identical on all 8 cores)."""
    import concourse.bass as bass
    import concourse.mybir as mybir
    from concourse.tile import TileContext

    _install_compat_patches()

    f32 = mybir.dt.float32
    bf16 = mybir.dt.bfloat16
    AFT = mybir.ActivationFunctionType

    from concourse import library_config

    nT = seq // 128       # T-tiles
    nQB = seq // QB       # attention q-blocks
    SW = seq // N_CORES   # AllToAll shard width (output rows per core)

    nc = bass.Bass("TRN2", target_bir_lowering=False, debug=False,
                   num_devices=N_CORES)
    x_d = nc.dram_tensor("x", [seq, D_MODEL], f32, kind="ExternalInput").ap()
    wq_d = nc.dram_tensor("w_slice", [D_MODEL, 3 * CPC], f32,
                          kind="ExternalInput").ap()
    wp_d = nc.dram_tensor("w_proj", [D_MODEL, D_MODEL], f32,
                          kind="ExternalInput").ap()
    id_d = nc.dram_tensor("ident", [128, 128], bf16, kind="ExternalInput").ap()
    mk_d = nc.dram_tensor("masks", [4, 128, QB], bf16,
                          kind="ExternalInput").ap()
    out_d = nc.dram_tensor("out", [SW, D_MODEL], f32,
                           kind="ExternalOutput").ap()

    with TileContext(nc) as tc:
        with (
            tc.tile_pool(name="per", bufs=1) as per,
            tc.tile_pool(name="stg", bufs=2) as stg,
            tc.tile_pool(name="dram", bufs=1, space="DRAM") as dram,
        ):
            qT = per.tile([128, seq], bf16)      # [2 heads x 64 d, T]
            kT = per.tile([128, seq], bf16)
            Vp = per.tile([128, nT, 130], bf16)  # V' tiles: [v_h0|1|v_h1|1]
            yn0 = per.tile([64, seq], bf16)      # normalized y^T, head 0
            yn1 = per.tile([64, seq], bf16)
            wqkv = per.tile([128, 8, 3 * CPC], bf16)
            wpj = per.tile([128, 8, D_MODEL], bf16)
            iden = per.tile([128, 128], bf16)
            mks = per.tile([128, 4, QB], bf16)
            ones = per.tile([128, 64], f32)
            a2a_sb = per.tile([128, 8, SW], bf16)

            nc.sync.dma_start(iden[:], id_d[:])
            for m in range(4):
                nc.sync.dma_start(mks[:, m, :], mk_d[m])
            nc.any.memset(ones[:], 1.0)
            nc.any.memset(Vp[:, :, 64:65], 1.0)
            nc.any.memset(Vp[:, :, 129:130], 1.0)

            # (weight staging happens inside the xstg pool below)

            dnd = dram.tile([16, QB], mybir.dt.float32)
            a2a_in = dram.tile([N_CORES * CPC, SW], bf16)
            a2a_out = dram.tile([N_CORES * CPC, SW], bf16)

            # ---- phases 0-2, braided emission ------------------------
            # Engines execute their scheduled streams in static order, so
            # overlap must be built into emission order: the prep work
            # (x-load/transpose/qkv/V') for block n+1 is interleaved chunk-
            # by-chunk between the attention groups of q-block n.  Attention
            # qb=n depends only on qkv blocks 0..n, so each braid is legal.
            # PSUM banks: pA 2x1 + sT 2x2 + yt0 1 + yt1 1 = 8
            with (
                tc.tile_pool(name="xp", bufs=1) as xp,
                tc.tile_pool(name="xstg", bufs=3) as xstg,
                tc.tile_pool(name="ps", bufs=2, space="PSUM") as ps,
            ):
                xT = xp.tile([128, 8, seq], bf16)   # [c-chunk part, chunk, T]

                def wqkv_stage():
                    for k in range(8):
                        wtmp = xstg.tile([128, 3 * CPC], f32, tag="xf",
                                         bufs=3, name=f"wtmp_{k}")
                        nc.sync.dma_start(wtmp[:],
                                          wq_d[128 * k:128 * (k + 1), :])
                        nc.vector.tensor_copy(wqkv[:, k, :], wtmp[:])

                def prep_chunks(n):
                    """Emit-closures for block n: loads, x^T, qkv^T, V'."""
                    state = {}

                    def loads():
                        xbs = []
                        for u in range(4):
                            t = 4 * n + u
                            xf = xstg.tile([128, D_MODEL], f32, tag="xf",
                                           bufs=3, name=f"xf_{t}")
                            nc.sync.dma_start(xf[:],
                                              x_d[128 * t:128 * (t + 1), :])
                            xb = xstg.tile([128, D_MODEL], bf16, tag="xb",
                                           bufs=4, name=f"xb_{t}")
                            nc.gpsimd.tensor_copy(xb[:], xf[:])
                            xbs.append(xb)
                        state["xbs"] = xbs

                    def xtr(u):
                        def emit():
                            t = 4 * n + u
                            xc = xstg.tile([128, 8, 128], bf16, tag="xc",
                                           bufs=3, name=f"xc_{t}")
                            nc.scalar.dma_start_transpose(
                                xc[:], state["xbs"][u][:])
                            nc.vector.tensor_copy(
                                xT[:, :, 128 * t:128 * (t + 1)], xc[:])
                        return emit

                    def trans(j):
                        # j indexes (x-tile u = j//2, c-chunk quad a = j%2):
                        # one PSUM tile holds 4 c-chunk transposes of a
                        # single x-tile, so work starts after its one load
                        def emit():
                            u, a = divmod(j, 2)
                            tp = ps.tile([128, 512], f32, tag="pA",
                                         name=f"tp_{n}_{j}")
                            for c in range(4):
                                nc.tensor.matmul(
                                    tp[:, 128 * c:128 * (c + 1)],
                                    state["xbs"][u][:, 128 * (4 * a + c):
                                                    128 * (4 * a + c + 1)],
                                    iden[:], start=True, stop=True)
                            nc.vector.tensor_copy(
                                xT[:, 4 * a:4 * (a + 1),
                                   128 * (4 * n + u):128 * (4 * n + u + 1)],
                                tp[:])
                        return emit

                    def qkv(m):
                        def emit():
                            qp = ps.tile([128, 512], f32, tag="pA",
                                         name=f"qp_{n}_{m}")
                            for k in range(8):
                                nc.tensor.matmul(
                                    qp[:],
                                    wqkv[:, k, 128 * m:128 * (m + 1)],
                                    xT[:, k, 512 * n:512 * (n + 1)],
                                    start=(k == 0), stop=(k == 7))
                            if m == 0:
                                nc.vector.tensor_copy(
                                    qT[:, 512 * n:512 * (n + 1)], qp[:])
                            elif m == 1:
                                nc.vector.tensor_copy(
                                    kT[:, 512 * n:512 * (n + 1)], qp[:])
                            else:
                                vs = xstg.tile([128, 512], bf16, tag="vs",
                                               bufs=2, name=f"vs_{n}")
                                nc.vector.tensor_copy(vs[:], qp[:])
                                state["vs"] = vs
                        return emit

                    def vtr(u):
                        def emit():
                            t = 4 * n + u
                            vs = state["vs"]
                            # separate PSUM tiles per head: PE-write plus
                            # DVE-read of one PSUM bank is a HW fault
                            vp0 = ps.tile([128, 64], f32, tag="pA",
                                          name=f"vp0_{t}")
                            vp1 = ps.tile([128, 64], f32, tag="pA",
                                          name=f"vp1_{t}")
                            nc.tensor.matmul(
                                vp0[:], vs[0:64, 128 * u:128 * (u + 1)],
                                iden[0:64, 0:64], start=True, stop=True)
                            nc.tensor.matmul(
                                vp1[:], vs[64:128, 128 * u:128 * (u + 1)],
                                iden[64:128, 64:128], start=True, stop=True)
                            nc.vector.tensor_copy(Vp[:, t, 0:64], vp0[:])
                            nc.vector.tensor_copy(Vp[:, t, 65:129], vp1[:])
                        return emit

                    if n >= XBAR_FROM_BLOCK:
                        return ([loads] + [xtr(u) for u in range(4)]
                                + [qkv(m) for m in range(3)]
                                + [vtr(u) for u in range(4)])
                    return ([loads] + [trans(j) for j in range(8)]
                            + [qkv(m) for m in range(3)]
                            + [vtr(u) for u in range(4)])

                def attention_groups(qb, ytps):
                    nkt = 4 * (qb + 1)

                    def group(g):
                        # diagonal k-tiles (d = kt-4qb >= 0) only attend to
                        # q >= 128d: trim score MM / exp / mask / yT MM to
                        # the valid column range [128d, QB).  q-cols below
                        # that are fully masked and, because kt=0 always
                        # covers the full width with start=True, never read.
                        def off(kt):
                            d = kt - 4 * qb
                            return 128 * d if d >= 0 else 0

                        def emit():
                            # h-inner MM order: consecutive score matmuls use
                            # disjoint PE row-groups (h0 rows 0-63, h1 rows
                            # 64-127) so the 16x32x32-subarray PE overlaps
                            # them (K=64 packing, ~2x on the score matmuls)
                            sps = [ps.tile([128, 2 * QB], f32, tag="sT",
                                           name=f"sp_{qb}_{g}_{h}")
                                   for h in (0, 1)]
                            for u in (0, 1):
                                kt = 2 * g + u
                                o = off(kt)
                                for h in (0, 1):
                                    nc.tensor.matmul(
                                        sps[h][:, QB * u + o:QB * (u + 1)],
                                        kT[64 * h:64 * (h + 1),
                                           128 * kt:128 * (kt + 1)],
                                        qT[64 * h:64 * (h + 1),
                                           QB * qb + o:QB * (qb + 1)],
                                        start=True, stop=True)
                            diag = off(2 * g) > 0 or off(2 * g + 1) > 0
                            for h in (0, 1):
                                pt = stg.tile([128, 2 * QB], bf16, tag="pT",
                                              bufs=3, name=f"pt_{qb}_{g}_{h}")
                                if diag:
                                    for u in (0, 1):
                                        o = off(2 * g + u)
                                        nc.scalar.activation(
                                            pt[:, QB * u + o:QB * (u + 1)],
                                            sps[h][:, QB * u + o:QB * (u + 1)],
                                            AFT.Exp, scale=0.125)
                                else:
                                    nc.scalar.activation(pt[:], sps[h][:],
                                                         AFT.Exp, scale=0.125)
                                for u in (0, 1):
                                    kt = 2 * g + u
                                    d = kt - 4 * qb
                                    o = off(kt)
                                    if d >= 0:
                                        w = min(o + 128, QB)
                                        nc.vector.tensor_mul(
                                            pt[:, QB * u + o:QB * u + w],
                                            pt[:, QB * u + o:QB * u + w],
                                            mks[:, d, o:w])
                                    nc.tensor.matmul(
                                        ytps[h][:, o:QB],
                                        Vp[:, kt, 65 * h:65 * (h + 1)],
                                        pt[:, QB * u + o:QB * (u + 1)],
                                        start=(kt == 0),
                                        stop=(kt == nkt - 1))
                        return emit

                    return [group(g) for g in range(nkt // 2)]

                def normalize(qb, ytps):
                    for h in (0, 1):
                        # one copy frees the PSUM accumulator right away; the
                        # denom -> 1/denom -> broadcast -> scale chain then
                        # runs from SBUF off the critical path.
                        yu = stg.tile([65, 2 * QB], f32, tag="dn", bufs=4,
                                      name=f"yu_{qb}_{h}")
                        nc.vector.tensor_copy(yu[:, 0:QB], ytps[h][:])
                        nc.scalar.activation(yu[64:65, QB:2 * QB],
                                             yu[64:65, 0:QB], AFT.Ln)
                        nc.scalar.activation(yu[64:65, QB:2 * QB],
                                             yu[64:65, QB:2 * QB], AFT.Exp,
                                             scale=-1.0)
                        yn = yn0 if h == 0 else yn1
                        if PBC_NORM and qb < nQB - 1:
                            bcs = stg.tile([64, QB], f32, tag="bc2", bufs=2,
                                           name=f"bcs_{qb}_{h}")
                            dslot = dnd[2 * qb + h:2 * qb + h + 1, :]
                            nc.sync.dma_start(dslot, yu[64:65, QB:2 * QB])
                            nc.sync.dma_start(
                                bcs[:], dslot.broadcast_to([64, QB]))
                            nc.vector.tensor_mul(
                                yn[:, QB * qb:QB * (qb + 1)],
                                yu[0:64, 0:QB], bcs[:])
                        else:
                            bcp = ps.tile([64, QB], f32, tag="pA",
                                          name=f"bcp_{qb}_{h}")
                            nc.tensor.matmul(bcp[:], ones[64:65, 0:64],
                                             yu[64:65, QB:2 * QB],
                                             start=True, stop=True)
                            nc.vector.tensor_mul(
                                yn[:, QB * qb:QB * (qb + 1)],
                                yu[0:64, 0:QB], bcp[:])
                        if SW == QB:
                            # q-block == shard: stage its AllToAll rows now
                            j = qb
                            r0 = 128 * j + 64 * h
                            nc.sync.dma_start(a2a_in[r0:r0 + 64, :],
                                              yn[:, SW * j:SW * (j + 1)])

                def wpj_chunk(k):
                    def emit():
                        # w_proj staged late (projection tail only) and
                        # braided into the final attention block, which has
                        # no other prep work to overlap with
                        ptmp = xstg.tile([128, D_MODEL], f32, tag="xf",
                                         bufs=3, name=f"ptmp_{k}")
                        nc.sync.dma_start(ptmp[:],
                                          wp_d[128 * k:128 * (k + 1), :])
                        nc.gpsimd.tensor_copy(wpj[:, k, :], ptmp[:])
                    return emit

                p0 = prep_chunks(0)
                p0[0]()           # stage-0 x loads lead the DMA queues
                wqkv_stage()
                for c in p0[1:]:
                    c()
                for n in range(nQB):
                    ytps = [ps.tile([65, QB], f32, tag=f"yt{h}", bufs=1,
                                    name=f"yt{h}_{n}") for h in (0, 1)]
                    if n + 1 < nQB:
                        pend = prep_chunks(n + 1)
                    else:
                        pend = [wpj_chunk(k) for k in range(8)]
                    groups = attention_groups(n, ytps)
                    ci = 0
                    for gi, g in enumerate(groups):
                        g()
                        want = (gi + 1) * len(pend) // len(groups)
                        while ci < want:
                            pend[ci]()
                            ci += 1
                    while ci < len(pend):
                        pend[ci]()
                        ci += 1
                    normalize(n, ytps)

            # ---- phase 3: AllToAll head-shard -> seq-shard ----------------
            if SW != QB:
                for j in range(N_CORES):
                    nc.sync.dma_start(a2a_in[128 * j:128 * j + 64, :],
                                      yn0[:, SW * j:SW * (j + 1)])
                    nc.sync.dma_start(a2a_in[128 * j + 64:128 * (j + 1), :],
                                      yn1[:, SW * j:SW * (j + 1)])
            if use_collective:
                nc.gpsimd.collective_compute(
                    "AllToAll", mybir.AluOpType.bypass,
                    ins=[a2a_in.opt()], outs=[a2a_out.opt()],
                    replica_groups=[list(range(N_CORES))])
            else:
                # timing-model variant (TimelineSim can't simulate
                # collectives): stand-in DRAM->DRAM copy
                nc.sync.dma_start(a2a_out[:], a2a_in[:])
            for j in range(N_CORES):
                nc.sync.dma_start(a2a_sb[:, j, :],
                                  a2a_out[128 * j:128 * (j + 1), :])

            # ---- phase 4: output projection for this core's SW rows -------
            with tc.tile_pool(name="psC", bufs=2, space="PSUM") as psC:
                mw = min(128, SW)
                for m in range(SW // mw):
                    pp = psC.tile([mw, D_MODEL], f32, tag="pp")
                    for n2 in (0, 1):
                        for k in range(8):
                            nc.tensor.matmul(
                                pp[:, 512 * n2:512 * (n2 + 1)],
                                a2a_sb[:, k, mw * m:mw * (m + 1)],
                                wpj[:, k, 512 * n2:512 * (n2 + 1)],
                                start=(k == 0), stop=(k == 7))
                    ob = stg.tile([mw, D_MODEL], f32, tag="ob", bufs=2)
                    nc.vector.tensor_copy(ob[:], pp[:])
                    nc.sync.dma_start(out_d[mw * m:mw * (m + 1), :], ob[:])

    if split_waits:
        _split_multi_waits(nc)
    return nc


def make_aux_inputs():
    ident = np.eye(128, dtype=BF16)
    k_idx = np.arange(128)[:, None]
    q_idx = np.arange(QB)[None, :]
    masks = np.stack(
        [((k_idx + 128 * d) <= q_idx).astype(BF16) for d in range(4)], axis=0)
    return ident, masks


def make_in_maps(x, w_qkv, w_proj, seq=SEQ):
    x = np.asarray(x, dtype=np.float32).reshape(seq, D_MODEL)
    w_qkv = np.asarray(w_qkv, dtype=np.float32)
    w_proj = np.asarray(w_proj, dtype=np.float32)
    ident, masks = make_aux_inputs()
    in_maps = []
    for i in range(N_CORES):
        sl = slice(CPC * i, CPC * (i + 1))
        w_slice = np.concatenate(
            [w_qkv[:, sl], w_qkv[:, D_MODEL:][:, sl],
             w_qkv[:, 2 * D_MODEL:][:, sl]], axis=1)
        in_maps.append({
            "x": x,
            "w_slice": np.ascontiguousarray(w_slice),
            "w_proj": w_proj,
            "ident": ident,
            "masks": masks,
        })
    return in_maps


_NC_CACHE = {}


def kernel(x, w_qkv, w_proj):
    """Full inputs in, full output out. Shards internally across 8 cores."""
    try:
        import os
        import jax
        jax.config.update("jax_compilation_cache_dir",
                          os.path.expanduser("~/.cache/jax_bass_kernel"))
        jax.config.update("jax_persistent_cache_min_compile_time_secs", 0.0)
    except Exception:
        pass
    from concourse.bass_utils import run_bass_kernel_spmd

    x = np.asarray(x, dtype=np.float32)
    batch = x.shape[0]
    seq = x.shape[1]
    if seq not in _NC_CACHE:
        _NC_CACHE[seq] = build_nc(seq)
    nc = _NC_CACHE[seq]
    in_maps = make_in_maps(x, w_qkv, w_proj, seq=seq)
    res = run_bass_kernel_spmd(nc, in_maps, list(range(N_CORES)))
    out = np.concatenate([res.results[j]["out"] for j in range(N_CORES)],
                         axis=0)
    return out.reshape(batch, seq, D_MODEL).astype(np.float32)



# revision 27
# speedup vs baseline: 1.0745x; 1.0309x over previous
"""Causal self-attention (d_model=1024, n_head=16, seq=4096) on 8 trn2 cores.

Sharding: tensor-parallel over heads (2 heads/core) for QKV + attention,
then an AllToAll re-shards y^T from head-sharded to sequence-sharded, so
each core runs the output projection for seq/8 rows with the full w_proj
(no AllReduce). The host concatenates the 8 row-shards.

Per-core layout (bf16 into the PE, fp32 PSUM accumulation):
  - x^T built via PE identity-matmul transposes (the d_model contraction
    needs x in [c, T] layout for both qkv operands).
  - qkv^T = w_slice.T @ x^T lands directly in [chan, T] layout, so qT/kT
    are exactly the lhsT/rhs of the score matmul (scores^T = K Q^T), and
    V' (normal orientation + a ones column) comes from small PE transposes.
  - softmax without max-subtraction (scores ~ N(0,1): exp cannot overflow
    fp32); the denominator falls out of the y^T matmul as the ones-column
    row; normalization uses exp(-ln(denom)) on ScalarE plus a K=1 matmul
    to broadcast the reciprocal across partitions.
  - causal masking: only lower-triangle k-tiles are computed; diagonal
    tiles are masked by a precomputed 0/1 multiply after the exp.
  - emission is braided: prep for block n+1 (x load/transpose/qkv/V') is
    interleaved between the attention groups of q-block n, under a single
    shared PSUM pool, so PE/ACT/DVE/DMA overlap across phases.
"""

import sys
import types

import numpy as np
import ml_dtypes

D_MODEL = 1024
N_HEAD = 16
SEQ = 4096
N_CORES = 8
D_HEAD = 64
CPC = 128            # channels per core (2 heads x 64)
QB = 512             # attention q-block width
BF16 = ml_dtypes.bfloat16
XBAR_FROM_BLOCK = 99   # blocks >= this: x^T via ACT-queue xbar into scratch
PBC_NORM = True       # normalize broadcast via gpsimd partition_broadcast


def _install_compat_patches():
    """Stub antenv.axon_hooks (absent in this container) so
    run_bass_kernel_spmd's trace path degrades instead of ImportError."""
    if "antenv.axon_hooks" not in sys.modules:
        mod = types.ModuleType("antenv.axon_hooks")
        mod.get_axon_ntff_profile_hook = lambda: None
        sys.modules["antenv.axon_hooks"] = mod


def _split_multi_waits(nc):
    """The nix walrus here accepts at most ONE sync-wait per instruction
    (setupSyncWait: 'Too many sync wait commands').  Hoist extra waits onto
    same-engine NoOps inserted immediately before the instruction — engine
    streams execute in program order, so semantics are unchanged."""
    import concourse.mybir as mybir

    n = 0
    for fn in nc.m.functions:
        for bb in fn.blocks:
            insts = bb.instructions
            out = []
            for inst in insts:
                si = getattr(inst, "sync_info", None)
                waits = list(si.on_wait) if si is not None else []
                if len(waits) > 1:
                    si.on_wait.clear()
                    for w in waits[:-1]:
                        n += 1
                        nop = mybir.InstNoOp(name=f"I-WSPLIT{n}", ins=[], outs=[])
                        nop.engine = inst.engine
                        nop.sync_info = mybir.SyncInfo(on_wait=[w], on_update=[])
                        out.append(nop)
                    si.on_wait.append(waits[-1])
                out.append(inst)
            bb.instructions = out


def build_nc(seq=SEQ, use_collective=True, split_waits=True):
    """Build the single-core SPMD program (identical on all 8 cores)."""
    import concourse.bass as bass
    import concourse.mybir as mybir
    from concourse.tile import TileContext

    _install_compat_patches()

    f32 = mybir.dt.float32
    bf16 = mybir.dt.bfloat16
    AFT = mybir.ActivationFunctionType

    from concourse import library_config

    nT = seq // 128       # T-tiles
    nQB = seq // QB       # attention q-blocks
    SW = seq // N_CORES   # AllToAll shard width (output rows per core)

    nc = bass.Bass("TRN2", target_bir_lowering=False, debug=False,
                   num_devices=N_CORES)
    x_d = nc.dram_tensor("x", [seq, D_MODEL], f32, kind="ExternalInput").ap()
    wq_d = nc.dram_tensor("w_slice", [D_MODEL, 3 * CPC], f32,
                          kind="ExternalInput").ap()
    wp_d = nc.dram_tensor("w_proj", [D_MODEL, D_MODEL], f32,
                          kind="ExternalInput").ap()
    id_d = nc.dram_tensor("ident", [128, 128], bf16, kind="ExternalInput").ap()
    mk_d = nc.dram_tensor("masks", [4, 128, QB], bf16,
                          kind="ExternalInput").ap()
    out_d = nc.dram_tensor("out", [SW, D_MODEL], f32,
                           kind="ExternalOutput").ap()

    with TileContext(nc) as tc:
        with (
            tc.tile_pool(name="per", bufs=1) as per,
            tc.tile_pool(name="stg", bufs=2) as stg,
            tc.tile_pool(name="dram", bufs=1, space="DRAM") as dram,
        ):
            qT = per.tile([128, seq], bf16)      # [2 heads x 64 d, T]
            kT = per.tile([128, seq], bf16)
            Vp = per.tile([128, nT, 130], bf16)  # V' tiles: [v_h0|1|v_h1|1]
            yn0 = per.tile([64, seq], bf16)      # normalized y^T, head 0
            yn1 = per.tile([64, seq], bf16)
            wqkv = per.tile([128, 8, 3 * CPC], bf16)
            wpj = per.tile([128, 8, D_MODEL], bf16)
            iden = per.tile([128, 128], bf16)
            mks = per.tile([128, 4, QB], bf16)
            ones = per.tile([128, 64], f32)
            a2a_sb = per.tile([128, 8, SW], bf16)

            nc.sync.dma_start(iden[:], id_d[:])
            for m in range(4):
                nc.sync.dma_start(mks[:, m, :], mk_d[m])
            nc.any.memset(ones[:], 1.0)
            nc.any.memset(Vp[:, :, 64:65], 1.0)
            nc.any.memset(Vp[:, :, 129:130], 1.0)

            # (weight staging happens inside the xstg pool below)

            dnd = dram.tile([16, QB], mybir.dt.float32)
            a2a_in = dram.tile([N_CORES * CPC, SW], bf16)
            a2a_out = dram.tile([N_CORES * CPC, SW], bf16)

            # ---- phases 0-2, braided emission ------------------------
            # Engines execute their scheduled streams in static order, so
            # overlap must be built into emission order: the prep work
            # (x-load/transpose/qkv/V') for block n+1 is interleaved chunk-
            # by-chunk between the attention groups of q-block n.  Attention
            # qb=n depends only on qkv blocks 0..n, so each braid is legal.
            # PSUM banks: pA 2x1 + sT 2x2 + yt0 1 + yt1 1 = 8
            with (
                tc.tile_pool(name="xp", bufs=1) as xp,
                tc.tile_pool(name="xstg", bufs=3) as xstg,
                tc.tile_pool(name="ps", bufs=2, space="PSUM") as ps,
            ):
                xT = xp.tile([128, 8, seq], bf16)   # [c-chunk part, chunk, T]

                def wqkv_stage():
                    for k in range(8):
                        wtmp = xstg.tile([128, 3 * CPC], f32, tag="xf",
                                         bufs=3, name=f"wtmp_{k}")
                        nc.sync.dma_start(wtmp[:],
                                          wq_d[128 * k:128 * (k + 1), :])
                        nc.vector.tensor_copy(wqkv[:, k, :], wtmp[:])

                def prep_chunks(n):
                    """Emit-closures for block n: loads, x^T, qkv^T, V'."""
                    state = {}

                    def loads():
                        xbs = []
                        for u in range(4):
                            t = 4 * n + u
                            xf = xstg.tile([128, D_MODEL], f32, tag="xf",
                                           bufs=3, name=f"xf_{t}")
                            nc.sync.dma_start(xf[:],
                                              x_d[128 * t:128 * (t + 1), :])
                            xb = xstg.tile([128, D_MODEL], bf16, tag="xb",
                                           bufs=4, name=f"xb_{t}")
                            nc.gpsimd.tensor_copy(xb[:], xf[:])
                            xbs.append(xb)
                        state["xbs"] = xbs

                    def xtr(u):
                        def emit():
                            t = 4 * n + u
                            xc = xstg.tile([128, 8, 128], bf16, tag="xc",
                                           bufs=3, name=f"xc_{t}")
                            nc.scalar.dma_start_transpose(
                                xc[:], state["xbs"][u][:])
                            nc.vector.tensor_copy(
                                xT[:, :, 128 * t:128 * (t + 1)], xc[:])
                        return emit

                    def trans(j):
                        # j indexes (x-tile u = j//2, c-chunk quad a = j%2):
                        # one PSUM tile holds 4 c-chunk transposes of a
                        # single x-tile, so work starts after its one load
                        def emit():
                            u, a = divmod(j, 2)
                            tp = ps.tile([128, 512], f32, tag="pA",
                                         name=f"tp_{n}_{j}")
                            for c in range(4):
                                nc.tensor.matmul(
                                    tp[:, 128 * c:128 * (c + 1)],
                                    state["xbs"][u][:, 128 * (4 * a + c):
                                                    128 * (4 * a + c + 1)],
                                    iden[:], start=True, stop=True)
                            nc.vector.tensor_copy(
                                xT[:, 4 * a:4 * (a + 1),
                                   128 * (4 * n + u):128 * (4 * n + u + 1)],
                                tp[:])
                        return emit

                    def qkv(m):
                        def emit():
                            qp = ps.tile([128, 512], f32, tag="pA",
                                         name=f"qp_{n}_{m}")
                            for k in range(8):
                                nc.tensor.matmul(
                                    qp[:],
                                    wqkv[:, k, 128 * m:128 * (m + 1)],
                                    xT[:, k, 512 * n:512 * (n + 1)],
                                    start=(k == 0), stop=(k == 7))
                            if m == 0:
                                nc.vector.tensor_copy(
                                    qT[:, 512 * n:512 * (n + 1)], qp[:])
                            elif m == 1:
                                nc.vector.tensor_copy(
                                    kT[:, 512 * n:512 * (n + 1)], qp[:])
                            else:
                                vs = xstg.tile([128, 512], bf16, tag="vs",
                                               bufs=2, name=f"vs_{n}")
                                nc.vector.tensor_copy(vs[:], qp[:])
                                state["vs"] = vs
                        return emit

                    def vtr(u):
                        def emit():
                            t = 4 * n + u
                            vs = state["vs"]
                            # separate PSUM tiles per head: PE-write plus
                            # DVE-read of one PSUM bank is a HW fault
                            vp0 = ps.tile([128, 64], f32, tag="pA",
                                          name=f"vp0_{t}")
                            vp1 = ps.tile([128, 64], f32, tag="pA",
                                          name=f"vp1_{t}")
                            nc.tensor.matmul(
                                vp0[:], vs[0:64, 128 * u:128 * (u + 1)],
                                iden[0:64, 0:64], start=True, stop=True)
                            nc.tensor.matmul(
                                vp1[:], vs[64:128, 128 * u:128 * (u + 1)],
                                iden[64:128, 64:128], start=True, stop=True)
                            nc.vector.tensor_copy(Vp[:, t, 0:64], vp0[:])
                            nc.vector.tensor_copy(Vp[:, t, 65:129], vp1[:])
                        return emit

                    if n >= XBAR_FROM_BLOCK:
                        return ([loads] + [xtr(u) for u in range(4)]
                                + [qkv(m) for m in range(3)]
                                + [vtr(u) for u in range(4)])
                    return ([loads] + [trans(j) for j in range(8)]
                            + [qkv(m) for m in range(3)]
                            + [vtr(u) for u in range(4)])

                def attention_groups(qb, ytps):
                    nkt = 4 * (qb + 1)

                    def group(g):
                        # diagonal k-tiles (d = kt-4qb >= 0) only attend to
                        # q >= 128d: trim score MM / exp / mask / yT MM to
                        # the valid column range [128d, QB).  q-cols below
                        # that are fully masked and, because kt=0 always
                        # covers the full width with start=True, never read.
                        def off(kt):
                            d = kt - 4 * qb
                            return 128 * d if d >= 0 else 0

                        def emit():
                            # h-inner MM order: consecutive score matmuls use
                            # disjoint PE row-groups (h0 rows 0-63, h1 rows
                            # 64-127) so the 16x32x32-subarray PE overlaps
                            # them (K=64 packing, ~2x on the score matmuls)
                            sps = [ps.tile([128, 2 * QB], f32, tag="sT",
                                           name=f"sp_{qb}_{g}_{h}")
                                   for h in (0, 1)]
                            for u in (0, 1):
                                kt = 2 * g + u
                                o = off(kt)
                                for h in (0, 1):
                                    nc.tensor.matmul(
                                        sps[h][:, QB * u + o:QB * (u + 1)],
                                        kT[64 * h:64 * (h + 1),
                                           128 * kt:128 * (kt + 1)],
                                        qT[64 * h:64 * (h + 1),
                                           QB * qb + o:QB * (qb + 1)],
                                        start=True, stop=True)
                            diag = off(2 * g) > 0 or off(2 * g + 1) > 0
                            for h in (0, 1):
                                pt = stg.tile([128, 2 * QB], bf16, tag="pT",
                                              bufs=3, name=f"pt_{qb}_{g}_{h}")
                                if diag:
                                    for u in (0, 1):
                                        o = off(2 * g + u)
                                        nc.scalar.activation(
                                            pt[:, QB * u + o:QB * (u + 1)],
                                            sps[h][:, QB * u + o:QB * (u + 1)],
                                            AFT.Exp, scale=0.125)
                                else:
                                    nc.scalar.activation(pt[:], sps[h][:],
                                                         AFT.Exp, scale=0.125)
                                for u in (0, 1):
                                    kt = 2 * g + u
                                    d = kt - 4 * qb
                                    o = off(kt)
                                    if d >= 0:
                                        w = min(o + 128, QB)
                                        nc.vector.tensor_mul(
                                            pt[:, QB * u + o:QB * u + w],
                                            pt[:, QB * u + o:QB * u + w],
                                            mks[:, d, o:w])
                                    nc.tensor.matmul(
                                        ytps[h][:, o:QB],
                                        Vp[:, kt, 65 * h:65 * (h + 1)],
                                        pt[:, QB * u + o:QB * (u + 1)],
                                        start=(kt == 0),
                                        stop=(kt == nkt - 1))
                        return emit

                    return [group(g) for g in range(nkt // 2)]

                def normalize(qb, ytps):
                    for h in (0, 1):
                        # one copy frees the PSUM accumulator right away; the
                        # denom -> 1/denom -> broadcast -> scale chain then
                        # runs from SBUF off the critical path.
                        yu = stg.tile([65, 2 * QB], f32, tag="dn", bufs=4,
                                      name=f"yu_{qb}_{h}")
                        nc.vector.tensor_copy(yu[:, 0:QB], ytps[h][:])
                        nc.vector.reciprocal(yu[64:65, QB:2 * QB],
                                             yu[64:65, 0:QB])
                        yn = yn0 if h == 0 else yn1
                        if PBC_NORM and qb < nQB - 1:
                            bcs = stg.tile([64, QB], f32, tag="bc2", bufs=2,
                                           name=f"bcs_{qb}_{h}")
                            dslot = dnd[2 * qb + h:2 * qb + h + 1, :]
                            nc.sync.dma_start(dslot, yu[64:65, QB:2 * QB])
                            nc.sync.dma_start(
                                bcs[:], dslot.broadcast_to([64, QB]))
                            nc.vector.tensor_mul(
                                yn[:, QB * qb:QB * (qb + 1)],
                                yu[0:64, 0:QB], bcs[:])
                        else:
                            bcp = ps.tile([64, QB], f32, tag="pA",
                                          name=f"bcp_{qb}_{h}")
                            nc.tensor.matmul(bcp[:], ones[64:65, 0:64],
                                             yu[64:65, QB:2 * QB],
                                             start=True, stop=True)
                            nc.vector.tensor_mul(
                                yn[:, QB * qb:QB * (qb + 1)],
                                yu[0:64, 0:QB], bcp[:])
                        if SW == QB:
                            # q-block == shard: stage its AllToAll rows now
                            j = qb
                            r0 = 128 * j + 64 * h
                            nc.sync.dma_start(a2a_in[r0:r0 + 64, :],
                                              yn[:, SW * j:SW * (j + 1)])

                def wpj_chunk(k):
                    def emit():
                        # w_proj staged late (projection tail only) and
                        # braided into the final attention block, which has
                        # no other prep work to overlap with
                        ptmp = xstg.tile([128, D_MODEL], f32, tag="xf",
                                         bufs=3, name=f"ptmp_{k}")
                        nc.sync.dma_start(ptmp[:],
                                          wp_d[128 * k:128 * (k + 1), :])
                        nc.gpsimd.tensor_copy(wpj[:, k, :], ptmp[:])
                    return emit

                p0 = prep_chunks(0)
                p0[0]()           # stage-0 x loads lead the DMA queues
                wqkv_stage()
                for c in p0[1:]:
                    c()
                for n in range(nQB):
                    ytps = [ps.tile([65, QB], f32, tag=f"yt{h}", bufs=1,
                                    name=f"yt{h}_{n}") for h in (0, 1)]
                    if n + 1 < nQB:
                        pend = prep_chunks(n + 1)
                    else:
                        pend = [wpj_chunk(k) for k in range(8)]
                    groups = attention_groups(n, ytps)
                    ci = 0
                    for gi, g in enumerate(groups):
                        g()
                        want = (gi + 1) * len(pend) // len(groups)
                        while ci < want:
                            pend[ci]()
                            ci += 1
                    while ci < len(pend):
                        pend[ci]()
                        ci += 1
                    normalize(n, ytps)

            # ---- phase 3: AllToAll head-shard -> seq-shard ----------------
            if SW != QB:
                for j in range(N_CORES):
                    nc.sync.dma_start(a2a_in[128 * j:128 * j + 64, :],
                                      yn0[:, SW * j:SW * (j + 1)])
                    nc.sync.dma_start(a2a_in[128 * j + 64:128 * (j + 1), :],
                                      yn1[:, SW * j:SW * (j + 1)])
            if use_collective:
                nc.gpsimd.collective_compute(
                    "AllToAll", mybir.AluOpType.bypass,
                    ins=[a2a_in.opt()], outs=[a2a_out.opt()],
                    replica_groups=[list(range(N_CORES))])
            else:
                # timing-model variant (TimelineSim can't simulate
                # collectives): stand-in DRAM->DRAM copy
                nc.sync.dma_start(a2a_out[:], a2a_in[:])
            for j in range(N_CORES):
                nc.sync.dma_start(a2a_sb[:, j, :],
                                  a2a_out[128 * j:128 * (j + 1), :])

            # ---- phase 4: output projection for this core's SW rows -------
            with tc.tile_pool(name="psC", bufs=2, space="PSUM") as psC:
                mw = min(128, SW)
                for m in range(SW // mw):
                    pp = psC.tile([mw, D_MODEL], f32, tag="pp")
                    for n2 in (0, 1):
                        for k in range(8):
                            nc.tensor.matmul(
                                pp[:, 512 * n2:512 * (n2 + 1)],
                                a2a_sb[:, k, mw * m:mw * (m + 1)],
                                wpj[:, k, 512 * n2:512 * (n2 + 1)],
                                start=(k == 0), stop=(k == 7))
                    ob = stg.tile([mw, D_MODEL], f32, tag="ob", bufs=2)
                    nc.vector.tensor_copy(ob[:], pp[:])
                    nc.sync.dma_start(out_d[mw * m:mw * (m + 1), :], ob[:])

    if split_waits:
        _split_multi_waits(nc)
    return nc


def make_aux_inputs():
    ident = np.eye(128, dtype=BF16)
    k_idx = np.arange(128)[:, None]
    q_idx = np.arange(QB)[None, :]
    masks = np.stack(
        [((k_idx + 128 * d) <= q_idx).astype(BF16) for d in range(4)], axis=0)
    return ident, masks


def make_in_maps(x, w_qkv, w_proj, seq=SEQ):
    x = np.asarray(x, dtype=np.float32).reshape(seq, D_MODEL)
    w_qkv = np.asarray(w_qkv, dtype=np.float32)
    w_proj = np.asarray(w_proj, dtype=np.float32)
    ident, masks = make_aux_inputs()
    in_maps = []
    for i in range(N_CORES):
        sl = slice(CPC * i, CPC * (i + 1))
        w_slice = np.concatenate(
            [w_qkv[:, sl], w_qkv[:, D_MODEL:][:, sl],
             w_qkv[:, 2 * D_MODEL:][:, sl]], axis=1)
        in_maps.append({
            "x": x,
            "w_slice": np.ascontiguousarray(w_slice),
            "w_proj": w_proj,
            "ident": ident,
            "masks": masks,
        })
    return in_maps


_NC_CACHE = {}


def kernel(x, w_qkv, w_proj):
    """Full inputs in, full output out. Shards internally across 8 cores."""
    try:
        import os
        import jax
        jax.config.update("jax_compilation_cache_dir",
                          os.path.expanduser("~/.cache/jax_bass_kernel"))
        jax.config.update("jax_persistent_cache_min_compile_time_secs", 0.0)
    except Exception:
        pass
    from concourse.bass_utils import run_bass_kernel_spmd

    x = np.asarray(x, dtype=np.float32)
    batch = x.shape[0]
    seq = x.shape[1]
    if seq not in _NC_CACHE:
        _NC_CACHE[seq] = build_nc(seq)
    nc = _NC_CACHE[seq]
    in_maps = make_in_maps(x, w_qkv, w_proj, seq=seq)
    res = run_bass_kernel_spmd(nc, in_maps, list(range(N_CORES)))
    out = np.concatenate([res.results[j]["out"] for j in range(N_CORES)],
                         axis=0)
    return out.reshape(batch, seq, D_MODEL).astype(np.float32)



# revision 28
# speedup vs baseline: 1.0806x; 1.0057x over previous
"""Causal self-attention (d_model=1024, n_head=16, seq=4096) on 8 trn2 cores.

Sharding: tensor-parallel over heads (2 heads/core) for QKV + attention,
then an AllToAll re-shards y^T from head-sharded to sequence-sharded, so
each core runs the output projection for seq/8 rows with the full w_proj
(no AllReduce). The host concatenates the 8 row-shards.

Per-core layout (bf16 into the PE, fp32 PSUM accumulation):
  - x^T built via PE identity-matmul transposes (the d_model contraction
    needs x in [c, T] layout for both qkv operands).
  - qkv^T = w_slice.T @ x^T lands directly in [chan, T] layout, so qT/kT
    are exactly the lhsT/rhs of the score matmul (scores^T = K Q^T), and
    V' (normal orientation + a ones column) comes from small PE transposes.
  - softmax without max-subtraction (scores ~ N(0,1): exp cannot overflow
    fp32); the denominator falls out of the y^T matmul as the ones-column
    row; normalization uses exp(-ln(denom)) on ScalarE plus a K=1 matmul
    to broadcast the reciprocal across partitions.
  - causal masking: only lower-triangle k-tiles are computed; diagonal
    tiles are masked by a precomputed 0/1 multiply after the exp.
  - emission is braided: prep for block n+1 (x load/transpose/qkv/V') is
    interleaved between the attention groups of q-block n, under a single
    shared PSUM pool, so PE/ACT/DVE/DMA overlap across phases.
"""

import sys
import types

import numpy as np
import ml_dtypes

D_MODEL = 1024
N_HEAD = 16
SEQ = 4096
N_CORES = 8
D_HEAD = 64
CPC = 128            # channels per core (2 heads x 64)
QB = 512             # attention q-block width
BF16 = ml_dtypes.bfloat16
XBAR_FROM_BLOCK = 99   # blocks >= this: x^T via ACT-queue xbar into scratch
PBC_NORM = True       # normalize broadcast via gpsimd partition_broadcast


def _install_compat_patches():
    """Stub antenv.axon_hooks (absent in this container) so
    run_bass_kernel_spmd's trace path degrades instead of ImportError."""
    if "antenv.axon_hooks" not in sys.modules:
        mod = types.ModuleType("antenv.axon_hooks")
        mod.get_axon_ntff_profile_hook = lambda: None
        sys.modules["antenv.axon_hooks"] = mod


def _split_multi_waits(nc):
    """The nix walrus here accepts at most ONE sync-wait per instruction
    (setupSyncWait: 'Too many sync wait commands').  Hoist extra waits onto
    same-engine NoOps inserted immediately before the instruction — engine
    streams execute in program order, so semantics are unchanged."""
    import concourse.mybir as mybir

    n = 0
    for fn in nc.m.functions:
        for bb in fn.blocks:
            insts = bb.instructions
            out = []
            for inst in insts:
                si = getattr(inst, "sync_info", None)
                waits = list(si.on_wait) if si is not None else []
                if len(waits) > 1:
                    si.on_wait.clear()
                    for w in waits[:-1]:
                        n += 1
                        nop = mybir.InstNoOp(name=f"I-WSPLIT{n}", ins=[], outs=[])
                        nop.engine = inst.engine
                        nop.sync_info = mybir.SyncInfo(on_wait=[w], on_update=[])
                        out.append(nop)
                    si.on_wait.append(waits[-1])
                out.append(inst)
            bb.instructions = out


def build_nc(seq=SEQ, use_collective=True, split_waits=True):
    """Build the single-core SPMD program (identical on all 8 cores)."""
    import concourse.bass as bass
    import concourse.mybir as mybir
    from concourse.tile import TileContext

    _install_compat_patches()

    f32 = mybir.dt.float32
    bf16 = mybir.dt.bfloat16
    AFT = mybir.ActivationFunctionType

    from concourse import library_config

    nT = seq // 128       # T-tiles
    nQB = seq // QB       # attention q-blocks
    SW = seq // N_CORES   # AllToAll shard width (output rows per core)

    nc = bass.Bass("TRN2", target_bir_lowering=False, debug=False,
                   num_devices=N_CORES)
    x_d = nc.dram_tensor("x", [seq, D_MODEL], f32, kind="ExternalInput").ap()
    wq_d = nc.dram_tensor("w_slice", [D_MODEL, 3 * CPC], f32,
                          kind="ExternalInput").ap()
    wp_d = nc.dram_tensor("w_proj", [D_MODEL, D_MODEL], f32,
                          kind="ExternalInput").ap()
    id_d = nc.dram_tensor("ident", [128, 128], bf16, kind="ExternalInput").ap()
    mk_d = nc.dram_tensor("masks", [4, 128, QB], bf16,
                          kind="ExternalInput").ap()
    out_d = nc.dram_tensor("out", [SW, D_MODEL], f32,
                           kind="ExternalOutput").ap()

    with TileContext(nc) as tc:
        with (
            tc.tile_pool(name="per", bufs=1) as per,
            tc.tile_pool(name="stg", bufs=2) as stg,
            tc.tile_pool(name="dram", bufs=1, space="DRAM") as dram,
        ):
            qT = per.tile([128, seq], bf16)      # [2 heads x 64 d, T]
            kT = per.tile([128, seq], bf16)
            Vp = per.tile([128, nT, 130], bf16)  # V' tiles: [v_h0|1|v_h1|1]
            yn0 = per.tile([64, seq], bf16)      # normalized y^T, head 0
            yn1 = per.tile([64, seq], bf16)
            wqkv = per.tile([128, 8, 3 * CPC], bf16)
            wpj = per.tile([128, 8, D_MODEL], bf16)
            iden = per.tile([128, 128], bf16)
            mks = per.tile([128, 4, QB], bf16)
            ones = per.tile([128, 64], f32)
            a2a_sb = per.tile([128, 8, SW], bf16)

            nc.sync.dma_start(iden[:], id_d[:])
            for m in range(4):
                nc.sync.dma_start(mks[:, m, :], mk_d[m])
            nc.any.memset(ones[:], 1.0)
            nc.any.memset(Vp[:, :, 64:65], 1.0)
            nc.any.memset(Vp[:, :, 129:130], 1.0)

            # (weight staging happens inside the xstg pool below)

            dnd = dram.tile([16, QB], mybir.dt.float32)
            a2a_in = dram.tile([N_CORES * CPC, SW], bf16)
            a2a_out = dram.tile([N_CORES * CPC, SW], bf16)

            # ---- phases 0-2, braided emission ------------------------
            # Engines execute their scheduled streams in static order, so
            # overlap must be built into emission order: the prep work
            # (x-load/transpose/qkv/V') for block n+1 is interleaved chunk-
            # by-chunk between the attention groups of q-block n.  Attention
            # qb=n depends only on qkv blocks 0..n, so each braid is legal.
            # PSUM banks: pA 2x1 + sT 2x2 + yt0 1 + yt1 1 = 8
            with (
                tc.tile_pool(name="xp", bufs=1) as xp,
                tc.tile_pool(name="xstg", bufs=3) as xstg,
                tc.tile_pool(name="ps", bufs=2, space="PSUM") as ps,
            ):
                xT = xp.tile([128, 8, seq], bf16)   # [c-chunk part, chunk, T]

                def wqkv_stage():
                    for k in range(8):
                        wtmp = xstg.tile([128, 3 * CPC], f32, tag="xf",
                                         bufs=3, name=f"wtmp_{k}")
                        nc.sync.dma_start(wtmp[:],
                                          wq_d[128 * k:128 * (k + 1), :])
                        nc.vector.tensor_copy(wqkv[:, k, :], wtmp[:])

                def prep_chunks(n):
                    """Emit-closures for block n: loads, x^T, qkv^T, V'."""
                    state = {}

                    def loads():
                        xbs = []
                        for u in range(4):
                            t = 4 * n + u
                            xf = xstg.tile([128, D_MODEL], f32, tag="xf",
                                           bufs=3, name=f"xf_{t}")
                            nc.sync.dma_start(xf[:],
                                              x_d[128 * t:128 * (t + 1), :])
                            xb = xstg.tile([128, D_MODEL], bf16, tag="xb",
                                           bufs=4, name=f"xb_{t}")
                            nc.gpsimd.tensor_copy(xb[:], xf[:])
                            xbs.append(xb)
                        state["xbs"] = xbs

                    def xtr(u):
                        def emit():
                            t = 4 * n + u
                            xc = xstg.tile([128, 8, 128], bf16, tag="xc",
                                           bufs=3, name=f"xc_{t}")
                            nc.scalar.dma_start_transpose(
                                xc[:], state["xbs"][u][:])
                            nc.vector.tensor_copy(
                                xT[:, :, 128 * t:128 * (t + 1)], xc[:])
                        return emit

                    def trans(j):
                        # j indexes (x-tile u = j//2, c-chunk quad a = j%2):
                        # one PSUM tile holds 4 c-chunk transposes of a
                        # single x-tile, so work starts after its one load
                        def emit():
                            u, a = divmod(j, 2)
                            tp = ps.tile([128, 512], f32, tag="pA",
                                         name=f"tp_{n}_{j}")
                            for c in range(4):
                                nc.tensor.matmul(
                                    tp[:, 128 * c:128 * (c + 1)],
                                    state["xbs"][u][:, 128 * (4 * a + c):
                                                    128 * (4 * a + c + 1)],
                                    iden[:], start=True, stop=True)
                            cpe = nc.vector if (j % 2 == 0) else nc.scalar
                            if cpe is nc.vector:
                                cpe.tensor_copy(
                                    xT[:, 4 * a:4 * (a + 1),
                                       128 * (4 * n + u):
                                       128 * (4 * n + u + 1)],
                                    tp[:])
                            else:
                                nc.scalar.copy(
                                    xT[:, 4 * a:4 * (a + 1),
                                       128 * (4 * n + u):
                                       128 * (4 * n + u + 1)],
                                    tp[:])
                        return emit

                    def qkv(m):
                        def emit():
                            qp = ps.tile([128, 512], f32, tag="pA",
                                         name=f"qp_{n}_{m}")
                            for k in range(8):
                                nc.tensor.matmul(
                                    qp[:],
                                    wqkv[:, k, 128 * m:128 * (m + 1)],
                                    xT[:, k, 512 * n:512 * (n + 1)],
                                    start=(k == 0), stop=(k == 7))
                            if m == 0:
                                nc.scalar.copy(
                                    qT[:, 512 * n:512 * (n + 1)], qp[:])
                            elif m == 1:
                                nc.scalar.copy(
                                    kT[:, 512 * n:512 * (n + 1)], qp[:])
                            else:
                                vs = xstg.tile([128, 512], bf16, tag="vs",
                                               bufs=2, name=f"vs_{n}")
                                nc.vector.tensor_copy(vs[:], qp[:])
                                state["vs"] = vs
                        return emit

                    def vtr(u):
                        def emit():
                            t = 4 * n + u
                            vs = state["vs"]
                            # separate PSUM tiles per head: PE-write plus
                            # DVE-read of one PSUM bank is a HW fault
                            vp0 = ps.tile([128, 64], f32, tag="pA",
                                          name=f"vp0_{t}")
                            vp1 = ps.tile([128, 64], f32, tag="pA",
                                          name=f"vp1_{t}")
                            nc.tensor.matmul(
                                vp0[:], vs[0:64, 128 * u:128 * (u + 1)],
                                iden[0:64, 0:64], start=True, stop=True)
                            nc.tensor.matmul(
                                vp1[:], vs[64:128, 128 * u:128 * (u + 1)],
                                iden[64:128, 64:128], start=True, stop=True)
                            nc.vector.tensor_copy(Vp[:, t, 0:64], vp0[:])
                            nc.vector.tensor_copy(Vp[:, t, 65:129], vp1[:])
                        return emit

                    if n >= XBAR_FROM_BLOCK:
                        return ([loads] + [xtr(u) for u in range(4)]
                                + [qkv(m) for m in range(3)]
                                + [vtr(u) for u in range(4)])
                    return ([loads] + [trans(j) for j in range(8)]
                            + [qkv(m) for m in range(3)]
                            + [vtr(u) for u in range(4)])

                def attention_groups(qb, ytps):
                    nkt = 4 * (qb + 1)

                    def group(g):
                        # diagonal k-tiles (d = kt-4qb >= 0) only attend to
                        # q >= 128d: trim score MM / exp / mask / yT MM to
                        # the valid column range [128d, QB).  q-cols below
                        # that are fully masked and, because kt=0 always
                        # covers the full width with start=True, never read.
                        def off(kt):
                            d = kt - 4 * qb
                            return 128 * d if d >= 0 else 0

                        def emit():
                            # h-inner MM order: consecutive score matmuls use
                            # disjoint PE row-groups (h0 rows 0-63, h1 rows
                            # 64-127) so the 16x32x32-subarray PE overlaps
                            # them (K=64 packing, ~2x on the score matmuls)
                            sps = [ps.tile([128, 2 * QB], f32, tag="sT",
                                           name=f"sp_{qb}_{g}_{h}")
                                   for h in (0, 1)]
                            for u in (0, 1):
                                kt = 2 * g + u
                                o = off(kt)
                                for h in (0, 1):
                                    nc.tensor.matmul(
                                        sps[h][:, QB * u + o:QB * (u + 1)],
                                        kT[64 * h:64 * (h + 1),
                                           128 * kt:128 * (kt + 1)],
                                        qT[64 * h:64 * (h + 1),
                                           QB * qb + o:QB * (qb + 1)],
                                        start=True, stop=True)
                            diag = off(2 * g) > 0 or off(2 * g + 1) > 0
                            for h in (0, 1):
                                pt = stg.tile([128, 2 * QB], bf16, tag="pT",
                                              bufs=3, name=f"pt_{qb}_{g}_{h}")
                                if diag:
                                    for u in (0, 1):
                                        o = off(2 * g + u)
                                        nc.scalar.activation(
                                            pt[:, QB * u + o:QB * (u + 1)],
                                            sps[h][:, QB * u + o:QB * (u + 1)],
                                            AFT.Exp, scale=0.125)
                                else:
                                    nc.scalar.activation(pt[:], sps[h][:],
                                                         AFT.Exp, scale=0.125)
                                for u in (0, 1):
                                    kt = 2 * g + u
                                    d = kt - 4 * qb
                                    o = off(kt)
                                    if d >= 0:
                                        w = min(o + 128, QB)
                                        nc.vector.tensor_mul(
                                            pt[:, QB * u + o:QB * u + w],
                                            pt[:, QB * u + o:QB * u + w],
                                            mks[:, d, o:w])
                                    nc.tensor.matmul(
                                        ytps[h][:, o:QB],
                                        Vp[:, kt, 65 * h:65 * (h + 1)],
                                        pt[:, QB * u + o:QB * (u + 1)],
                                        start=(kt == 0),
                                        stop=(kt == nkt - 1))
                        return emit

                    return [group(g) for g in range(nkt // 2)]

                def normalize(qb, ytps):
                    for h in (0, 1):
                        # one copy frees the PSUM accumulator right away; the
                        # denom -> 1/denom -> broadcast -> scale chain then
                        # runs from SBUF off the critical path.
                        yu = stg.tile([65, 2 * QB], f32, tag="dn", bufs=4,
                                      name=f"yu_{qb}_{h}")
                        nc.vector.tensor_copy(yu[:, 0:QB], ytps[h][:])
                        nc.vector.reciprocal(yu[64:65, QB:2 * QB],
                                             yu[64:65, 0:QB])
                        yn = yn0 if h == 0 else yn1
                        if PBC_NORM and qb < nQB - 1:
                            bcs = stg.tile([64, QB], f32, tag="bc2", bufs=2,
                                           name=f"bcs_{qb}_{h}")
                            dslot = dnd[2 * qb + h:2 * qb + h + 1, :]
                            nc.sync.dma_start(dslot, yu[64:65, QB:2 * QB])
                            nc.sync.dma_start(
                                bcs[:], dslot.broadcast_to([64, QB]))
                            nc.vector.tensor_mul(
                                yn[:, QB * qb:QB * (qb + 1)],
                                yu[0:64, 0:QB], bcs[:])
                        else:
                            bcp = ps.tile([64, QB], f32, tag="pA",
                                          name=f"bcp_{qb}_{h}")
                            nc.tensor.matmul(bcp[:], ones[64:65, 0:64],
                                             yu[64:65, QB:2 * QB],
                                             start=True, stop=True)
                            nc.vector.tensor_mul(
                                yn[:, QB * qb:QB * (qb + 1)],
                                yu[0:64, 0:QB], bcp[:])
                        if SW == QB:
                            # q-block == shard: stage its AllToAll rows now
                            j = qb
                            r0 = 128 * j + 64 * h
                            nc.sync.dma_start(a2a_in[r0:r0 + 64, :],
                                              yn[:, SW * j:SW * (j + 1)])

                def wpj_chunk(k):
                    def emit():
                        # w_proj staged late (projection tail only) and
                        # braided into the final attention block, which has
                        # no other prep work to overlap with
                        ptmp = xstg.tile([128, D_MODEL], f32, tag="xf",
                                         bufs=3, name=f"ptmp_{k}")
                        nc.sync.dma_start(ptmp[:],
                                          wp_d[128 * k:128 * (k + 1), :])
                        nc.gpsimd.tensor_copy(wpj[:, k, :], ptmp[:])
                    return emit

                p0 = prep_chunks(0)
                p0[0]()           # stage-0 x loads lead the DMA queues
                wqkv_stage()
                for c in p0[1:]:
                    c()
                for n in range(nQB):
                    ytps = [ps.tile([65, QB], f32, tag=f"yt{h}", bufs=1,
                                    name=f"yt{h}_{n}") for h in (0, 1)]
                    if n + 1 < nQB:
                        pend = prep_chunks(n + 1)
                    else:
                        pend = [wpj_chunk(k) for k in range(8)]
                    groups = attention_groups(n, ytps)
                    ci = 0
                    for gi, g in enumerate(groups):
                        g()
                        want = (gi + 1) * len(pend) // len(groups)
                        while ci < want:
                            pend[ci]()
                            ci += 1
                    while ci < len(pend):
                        pend[ci]()
                        ci += 1
                    normalize(n, ytps)

            # ---- phase 3: AllToAll head-shard -> seq-shard ----------------
            if SW != QB:
                for j in range(N_CORES):
                    nc.sync.dma_start(a2a_in[128 * j:128 * j + 64, :],
                                      yn0[:, SW * j:SW * (j + 1)])
                    nc.sync.dma_start(a2a_in[128 * j + 64:128 * (j + 1), :],
                                      yn1[:, SW * j:SW * (j + 1)])
            if use_collective:
                nc.gpsimd.collective_compute(
                    "AllToAll", mybir.AluOpType.bypass,
                    ins=[a2a_in.opt()], outs=[a2a_out.opt()],
                    replica_groups=[list(range(N_CORES))])
            else:
                # timing-model variant (TimelineSim can't simulate
                # collectives): stand-in DRAM->DRAM copy
                nc.sync.dma_start(a2a_out[:], a2a_in[:])
            for j in range(N_CORES):
                nc.sync.dma_start(a2a_sb[:, j, :],
                                  a2a_out[128 * j:128 * (j + 1), :])

            # ---- phase 4: output projection for this core's SW rows -------
            with tc.tile_pool(name="psC", bufs=2, space="PSUM") as psC:
                mw = min(128, SW)
                for m in range(SW // mw):
                    pp = psC.tile([mw, D_MODEL], f32, tag="pp")
                    for n2 in (0, 1):
                        for k in range(8):
                            nc.tensor.matmul(
                                pp[:, 512 * n2:512 * (n2 + 1)],
                                a2a_sb[:, k, mw * m:mw * (m + 1)],
                                wpj[:, k, 512 * n2:512 * (n2 + 1)],
                                start=(k == 0), stop=(k == 7))
                    ob = stg.tile([mw, D_MODEL], f32, tag="ob", bufs=2)
                    nc.vector.tensor_copy(ob[:], pp[:])
                    nc.sync.dma_start(out_d[mw * m:mw * (m + 1), :], ob[:])

    if split_waits:
        _split_multi_waits(nc)
    return nc


def make_aux_inputs():
    ident = np.eye(128, dtype=BF16)
    k_idx = np.arange(128)[:, None]
    q_idx = np.arange(QB)[None, :]
    masks = np.stack(
        [((k_idx + 128 * d) <= q_idx).astype(BF16) for d in range(4)], axis=0)
    return ident, masks


def make_in_maps(x, w_qkv, w_proj, seq=SEQ):
    x = np.asarray(x, dtype=np.float32).reshape(seq, D_MODEL)
    w_qkv = np.asarray(w_qkv, dtype=np.float32)
    w_proj = np.asarray(w_proj, dtype=np.float32)
    ident, masks = make_aux_inputs()
    in_maps = []
    for i in range(N_CORES):
        sl = slice(CPC * i, CPC * (i + 1))
        w_slice = np.concatenate(
            [w_qkv[:, sl], w_qkv[:, D_MODEL:][:, sl],
             w_qkv[:, 2 * D_MODEL:][:, sl]], axis=1)
        in_maps.append({
            "x": x,
            "w_slice": np.ascontiguousarray(w_slice),
            "w_proj": w_proj,
            "ident": ident,
            "masks": masks,
        })
    return in_maps


_NC_CACHE = {}


def kernel(x, w_qkv, w_proj):
    """Full inputs in, full output out. Shards internally across 8 cores."""
    try:
        import os
        import jax
        jax.config.update("jax_compilation_cache_dir",
                          os.path.expanduser("~/.cache/jax_bass_kernel"))
        jax.config.update("jax_persistent_cache_min_compile_time_secs", 0.0)
    except Exception:
        pass
    from concourse.bass_utils import run_bass_kernel_spmd

    x = np.asarray(x, dtype=np.float32)
    batch = x.shape[0]
    seq = x.shape[1]
    if seq not in _NC_CACHE:
        _NC_CACHE[seq] = build_nc(seq)
    nc = _NC_CACHE[seq]
    in_maps = make_in_maps(x, w_qkv, w_proj, seq=seq)
    res = run_bass_kernel_spmd(nc, in_maps, list(range(N_CORES)))
    out = np.concatenate([res.results[j]["out"] for j in range(N_CORES)],
                         axis=0)
    return out.reshape(batch, seq, D_MODEL).astype(np.float32)



# revision 29
# speedup vs baseline: 1.0866x; 1.0055x over previous
"""Causal self-attention (d_model=1024, n_head=16, seq=4096) on 8 trn2 cores.

Sharding: tensor-parallel over heads (2 heads/core) for QKV + attention,
then an AllToAll re-shards y^T from head-sharded to sequence-sharded, so
each core runs the output projection for seq/8 rows with the full w_proj
(no AllReduce). The host concatenates the 8 row-shards.

Per-core layout (bf16 into the PE, fp32 PSUM accumulation):
  - x^T built via PE identity-matmul transposes (the d_model contraction
    needs x in [c, T] layout for both qkv operands).
  - qkv^T = w_slice.T @ x^T lands directly in [chan, T] layout, so qT/kT
    are exactly the lhsT/rhs of the score matmul (scores^T = K Q^T), and
    V' (normal orientation + a ones column) comes from small PE transposes.
  - softmax without max-subtraction (scores ~ N(0,1): exp cannot overflow
    fp32); the denominator falls out of the y^T matmul as the ones-column
    row; normalization uses exp(-ln(denom)) on ScalarE plus a K=1 matmul
    to broadcast the reciprocal across partitions.
  - causal masking: only lower-triangle k-tiles are computed; diagonal
    tiles are masked by a precomputed 0/1 multiply after the exp.
  - emission is braided: prep for block n+1 (x load/transpose/qkv/V') is
    interleaved between the attention groups of q-block n, under a single
    shared PSUM pool, so PE/ACT/DVE/DMA overlap across phases.
"""

import sys
import types

import numpy as np
import ml_dtypes

D_MODEL = 1024
N_HEAD = 16
SEQ = 4096
N_CORES = 8
D_HEAD = 64
CPC = 128            # channels per core (2 heads x 64)
QB = 512             # attention q-block width
BF16 = ml_dtypes.bfloat16
XBAR_FROM_BLOCK = 99   # blocks >= this: x^T via ACT-queue xbar into scratch
PBC_NORM = True       # normalize broadcast via gpsimd partition_broadcast


def _install_compat_patches():
    """Stub antenv.axon_hooks (absent in this container) so
    run_bass_kernel_spmd's trace path degrades instead of ImportError."""
    if "antenv.axon_hooks" not in sys.modules:
        mod = types.ModuleType("antenv.axon_hooks")
        mod.get_axon_ntff_profile_hook = lambda: None
        sys.modules["antenv.axon_hooks"] = mod


def _split_multi_waits(nc):
    """The nix walrus here accepts at most ONE sync-wait per instruction
    (setupSyncWait: 'Too many sync wait commands').  Hoist extra waits onto
    same-engine NoOps inserted immediately before the instruction — engine
    streams execute in program order, so semantics are unchanged."""
    import concourse.mybir as mybir

    n = 0
    for fn in nc.m.functions:
        for bb in fn.blocks:
            insts = bb.instructions
            out = []
            for inst in insts:
                si = getattr(inst, "sync_info", None)
                waits = list(si.on_wait) if si is not None else []
                if len(waits) > 1:
                    si.on_wait.clear()
                    for w in waits[:-1]:
                        n += 1
                        nop = mybir.InstNoOp(name=f"I-WSPLIT{n}", ins=[], outs=[])
                        nop.engine = inst.engine
                        nop.sync_info = mybir.SyncInfo(on_wait=[w], on_update=[])
                        out.append(nop)
                    si.on_wait.append(waits[-1])
                out.append(inst)
            bb.instructions = out


def build_nc(seq=SEQ, use_collective=True, split_waits=True):
    """Build the single-core SPMD program (identical on all 8 cores)."""
    import concourse.bass as bass
    import concourse.mybir as mybir
    from concourse.tile import TileContext

    _install_compat_patches()

    f32 = mybir.dt.float32
    bf16 = mybir.dt.bfloat16
    AFT = mybir.ActivationFunctionType

    from concourse import library_config

    nT = seq // 128       # T-tiles
    nQB = seq // QB       # attention q-blocks
    SW = seq // N_CORES   # AllToAll shard width (output rows per core)

    nc = bass.Bass("TRN2", target_bir_lowering=False, debug=False,
                   num_devices=N_CORES)
    x_d = nc.dram_tensor("x", [seq, D_MODEL], f32, kind="ExternalInput").ap()
    wq_d = nc.dram_tensor("w_slice", [D_MODEL, 3 * CPC], f32,
                          kind="ExternalInput").ap()
    wp_d = nc.dram_tensor("w_proj", [D_MODEL, D_MODEL], f32,
                          kind="ExternalInput").ap()
    id_d = nc.dram_tensor("ident", [128, 128], bf16, kind="ExternalInput").ap()
    mk_d = nc.dram_tensor("masks", [4, 128, QB], bf16,
                          kind="ExternalInput").ap()
    out_d = nc.dram_tensor("out", [SW, D_MODEL], f32,
                           kind="ExternalOutput").ap()

    with TileContext(nc) as tc:
        with (
            tc.tile_pool(name="per", bufs=1) as per,
            tc.tile_pool(name="stg", bufs=2) as stg,
            tc.tile_pool(name="dram", bufs=1, space="DRAM") as dram,
        ):
            qT = per.tile([128, seq], bf16)      # [2 heads x 64 d, T]
            kT = per.tile([128, seq], bf16)
            Vp = per.tile([128, nT, 130], bf16)  # V' tiles: [v_h0|1|v_h1|1]
            yn0 = per.tile([64, seq], bf16)      # normalized y^T, head 0
            yn1 = per.tile([64, seq], bf16)
            wqkv = per.tile([128, 8, 3 * CPC], bf16)
            wpj = per.tile([128, 8, D_MODEL], bf16)
            iden = per.tile([128, 128], bf16)
            mks = per.tile([128, 4, QB], bf16)
            ones = per.tile([128, 64], f32)
            a2a_sb = per.tile([128, 8, SW], bf16)

            nc.sync.dma_start(iden[:], id_d[:])
            for m in range(4):
                nc.sync.dma_start(mks[:, m, :], mk_d[m])
            nc.any.memset(ones[:], 1.0)
            nc.any.memset(Vp[:, :, 64:65], 1.0)
            nc.any.memset(Vp[:, :, 129:130], 1.0)

            # (weight staging happens inside the xstg pool below)

            dnd = dram.tile([16, QB], mybir.dt.float32)
            a2a_in = dram.tile([N_CORES * CPC, SW], bf16)
            a2a_out = dram.tile([N_CORES * CPC, SW], bf16)

            # ---- phases 0-2, braided emission ------------------------
            # Engines execute their scheduled streams in static order, so
            # overlap must be built into emission order: the prep work
            # (x-load/transpose/qkv/V') for block n+1 is interleaved chunk-
            # by-chunk between the attention groups of q-block n.  Attention
            # qb=n depends only on qkv blocks 0..n, so each braid is legal.
            # PSUM banks: pA 2x1 + sT 2x2 + yt0 1 + yt1 1 = 8
            with (
                tc.tile_pool(name="xp", bufs=1) as xp,
                tc.tile_pool(name="xstg", bufs=3) as xstg,
                tc.tile_pool(name="ps", bufs=2, space="PSUM") as ps,
            ):
                xT = xp.tile([128, 8, seq], bf16)   # [c-chunk part, chunk, T]

                def wqkv_stage():
                    for k in range(8):
                        wtmp = xstg.tile([128, 3 * CPC], f32, tag="xf",
                                         bufs=4, name=f"wtmp_{k}")
                        nc.sync.dma_start(wtmp[:],
                                          wq_d[128 * k:128 * (k + 1), :])
                        nc.vector.tensor_copy(wqkv[:, k, :], wtmp[:])

                def prep_chunks(n):
                    """Emit-closures for block n: loads, x^T, qkv^T, V'."""
                    state = {}

                    def loads():
                        xbs = []
                        for u in range(4):
                            t = 4 * n + u
                            xf = xstg.tile([128, D_MODEL], f32, tag="xf",
                                           bufs=4, name=f"xf_{t}")
                            nc.sync.dma_start(xf[:],
                                              x_d[128 * t:128 * (t + 1), :])
                            xb = xstg.tile([128, D_MODEL], bf16, tag="xb",
                                           bufs=6, name=f"xb_{t}")
                            nc.gpsimd.tensor_copy(xb[:], xf[:])
                            xbs.append(xb)
                        state["xbs"] = xbs

                    def xtr(u):
                        def emit():
                            t = 4 * n + u
                            xc = xstg.tile([128, 8, 128], bf16, tag="xc",
                                           bufs=3, name=f"xc_{t}")
                            nc.scalar.dma_start_transpose(
                                xc[:], state["xbs"][u][:])
                            nc.vector.tensor_copy(
                                xT[:, :, 128 * t:128 * (t + 1)], xc[:])
                        return emit

                    def trans(j):
                        # j indexes (x-tile u = j//2, c-chunk quad a = j%2):
                        # one PSUM tile holds 4 c-chunk transposes of a
                        # single x-tile, so work starts after its one load
                        def emit():
                            u, a = divmod(j, 2)
                            tp = ps.tile([128, 512], f32, tag="pA",
                                         name=f"tp_{n}_{j}")
                            for c in range(4):
                                nc.tensor.matmul(
                                    tp[:, 128 * c:128 * (c + 1)],
                                    state["xbs"][u][:, 128 * (4 * a + c):
                                                    128 * (4 * a + c + 1)],
                                    iden[:], start=True, stop=True)
                            cpe = nc.vector if (j % 2 == 0) else nc.scalar
                            if cpe is nc.vector:
                                cpe.tensor_copy(
                                    xT[:, 4 * a:4 * (a + 1),
                                       128 * (4 * n + u):
                                       128 * (4 * n + u + 1)],
                                    tp[:])
                            else:
                                nc.scalar.copy(
                                    xT[:, 4 * a:4 * (a + 1),
                                       128 * (4 * n + u):
                                       128 * (4 * n + u + 1)],
                                    tp[:])
                        return emit

                    def qkv(m):
                        def emit():
                            qp = ps.tile([128, 512], f32, tag="pA",
                                         name=f"qp_{n}_{m}")
                            for k in range(8):
                                nc.tensor.matmul(
                                    qp[:],
                                    wqkv[:, k, 128 * m:128 * (m + 1)],
                                    xT[:, k, 512 * n:512 * (n + 1)],
                                    start=(k == 0), stop=(k == 7))
                            if m == 0:
                                nc.scalar.copy(
                                    qT[:, 512 * n:512 * (n + 1)], qp[:])
                            elif m == 1:
                                nc.scalar.copy(
                                    kT[:, 512 * n:512 * (n + 1)], qp[:])
                            else:
                                vs = xstg.tile([128, 512], bf16, tag="vs",
                                               bufs=2, name=f"vs_{n}")
                                nc.vector.tensor_copy(vs[:], qp[:])
                                state["vs"] = vs
                        return emit

                    def vtr(u):
                        def emit():
                            t = 4 * n + u
                            vs = state["vs"]
                            # separate PSUM tiles per head: PE-write plus
                            # DVE-read of one PSUM bank is a HW fault
                            vp0 = ps.tile([128, 64], f32, tag="pA",
                                          name=f"vp0_{t}")
                            vp1 = ps.tile([128, 64], f32, tag="pA",
                                          name=f"vp1_{t}")
                            nc.tensor.matmul(
                                vp0[:], vs[0:64, 128 * u:128 * (u + 1)],
                                iden[0:64, 0:64], start=True, stop=True)
                            nc.tensor.matmul(
                                vp1[:], vs[64:128, 128 * u:128 * (u + 1)],
                                iden[64:128, 64:128], start=True, stop=True)
                            nc.vector.tensor_copy(Vp[:, t, 0:64], vp0[:])
                            nc.vector.tensor_copy(Vp[:, t, 65:129], vp1[:])
                        return emit

                    if n >= XBAR_FROM_BLOCK:
                        return ([loads] + [xtr(u) for u in range(4)]
                                + [qkv(m) for m in range(3)]
                                + [vtr(u) for u in range(4)])
                    return ([loads] + [trans(j) for j in range(8)]
                            + [qkv(m) for m in range(3)]
                            + [vtr(u) for u in range(4)])

                def attention_groups(qb, ytps):
                    nkt = 4 * (qb + 1)

                    def group(g):
                        # diagonal k-tiles (d = kt-4qb >= 0) only attend to
                        # q >= 128d: trim score MM / exp / mask / yT MM to
                        # the valid column range [128d, QB).  q-cols below
                        # that are fully masked and, because kt=0 always
                        # covers the full width with start=True, never read.
                        def off(kt):
                            d = kt - 4 * qb
                            return 128 * d if d >= 0 else 0

                        def emit():
                            # h-inner MM order: consecutive score matmuls use
                            # disjoint PE row-groups (h0 rows 0-63, h1 rows
                            # 64-127) so the 16x32x32-subarray PE overlaps
                            # them (K=64 packing, ~2x on the score matmuls)
                            sps = [ps.tile([128, 2 * QB], f32, tag="sT",
                                           name=f"sp_{qb}_{g}_{h}")
                                   for h in (0, 1)]
                            for u in (0, 1):
                                kt = 2 * g + u
                                o = off(kt)
                                for h in (0, 1):
                                    nc.tensor.matmul(
                                        sps[h][:, QB * u + o:QB * (u + 1)],
                                        kT[64 * h:64 * (h + 1),
                                           128 * kt:128 * (kt + 1)],
                                        qT[64 * h:64 * (h + 1),
                                           QB * qb + o:QB * (qb + 1)],
                                        start=True, stop=True)
                            diag = off(2 * g) > 0 or off(2 * g + 1) > 0
                            for h in (0, 1):
                                pt = stg.tile([128, 2 * QB], bf16, tag="pT",
                                              bufs=3, name=f"pt_{qb}_{g}_{h}")
                                if diag:
                                    for u in (0, 1):
                                        o = off(2 * g + u)
                                        nc.scalar.activation(
                                            pt[:, QB * u + o:QB * (u + 1)],
                                            sps[h][:, QB * u + o:QB * (u + 1)],
                                            AFT.Exp, scale=0.125)
                                else:
                                    nc.scalar.activation(pt[:], sps[h][:],
                                                         AFT.Exp, scale=0.125)
                                for u in (0, 1):
                                    kt = 2 * g + u
                                    d = kt - 4 * qb
                                    o = off(kt)
                                    if d >= 0:
                                        w = min(o + 128, QB)
                                        nc.vector.tensor_mul(
                                            pt[:, QB * u + o:QB * u + w],
                                            pt[:, QB * u + o:QB * u + w],
                                            mks[:, d, o:w])
                                    nc.tensor.matmul(
                                        ytps[h][:, o:QB],
                                        Vp[:, kt, 65 * h:65 * (h + 1)],
                                        pt[:, QB * u + o:QB * (u + 1)],
                                        start=(kt == 0),
                                        stop=(kt == nkt - 1))
                        return emit

                    return [group(g) for g in range(nkt // 2)]

                def normalize(qb, ytps):
                    for h in (0, 1):
                        # one copy frees the PSUM accumulator right away; the
                        # denom -> 1/denom -> broadcast -> scale chain then
                        # runs from SBUF off the critical path.
                        yu = stg.tile([65, 2 * QB], f32, tag="dn", bufs=4,
                                      name=f"yu_{qb}_{h}")
                        nc.vector.tensor_copy(yu[:, 0:QB], ytps[h][:])
                        nc.vector.reciprocal(yu[64:65, QB:2 * QB],
                                             yu[64:65, 0:QB])
                        yn = yn0 if h == 0 else yn1
                        if PBC_NORM and qb < nQB - 1:
                            bcs = stg.tile([64, QB], f32, tag="bc2", bufs=2,
                                           name=f"bcs_{qb}_{h}")
                            dslot = dnd[2 * qb + h:2 * qb + h + 1, :]
                            nc.sync.dma_start(dslot, yu[64:65, QB:2 * QB])
                            nc.sync.dma_start(
                                bcs[:], dslot.broadcast_to([64, QB]))
                            nc.vector.tensor_mul(
                                yn[:, QB * qb:QB * (qb + 1)],
                                yu[0:64, 0:QB], bcs[:])
                        else:
                            bcp = ps.tile([64, QB], f32, tag="pA",
                                          name=f"bcp_{qb}_{h}")
                            nc.tensor.matmul(bcp[:], ones[64:65, 0:64],
                                             yu[64:65, QB:2 * QB],
                                             start=True, stop=True)
                            nc.vector.tensor_mul(
                                yn[:, QB * qb:QB * (qb + 1)],
                                yu[0:64, 0:QB], bcp[:])
                        if SW == QB:
                            # q-block == shard: stage its AllToAll rows now
                            j = qb
                            r0 = 128 * j + 64 * h
                            nc.sync.dma_start(a2a_in[r0:r0 + 64, :],
                                              yn[:, SW * j:SW * (j + 1)])

                def wpj_chunk(k):
                    def emit():
                        # w_proj staged late (projection tail only) and
                        # braided into the final attention block, which has
                        # no other prep work to overlap with
                        ptmp = xstg.tile([128, D_MODEL], f32, tag="xf",
                                         bufs=4, name=f"ptmp_{k}")
                        nc.sync.dma_start(ptmp[:],
                                          wp_d[128 * k:128 * (k + 1), :])
                        nc.gpsimd.tensor_copy(wpj[:, k, :], ptmp[:])
                    return emit

                p0 = prep_chunks(0)
                p0[0]()           # stage-0 x loads lead the DMA queues
                wqkv_stage()
                for c in p0[1:]:
                    c()
                for n in range(nQB):
                    ytps = [ps.tile([65, QB], f32, tag=f"yt{h}", bufs=1,
                                    name=f"yt{h}_{n}") for h in (0, 1)]
                    if n + 1 < nQB:
                        pend = prep_chunks(n + 1)
                    else:
                        pend = [wpj_chunk(k) for k in range(8)]
                    groups = attention_groups(n, ytps)
                    ci = 0
                    for gi, g in enumerate(groups):
                        g()
                        want = (gi + 1) * len(pend) // len(groups)
                        while ci < want:
                            pend[ci]()
                            ci += 1
                    while ci < len(pend):
                        pend[ci]()
                        ci += 1
                    normalize(n, ytps)

            # ---- phase 3: AllToAll head-shard -> seq-shard ----------------
            if SW != QB:
                for j in range(N_CORES):
                    nc.sync.dma_start(a2a_in[128 * j:128 * j + 64, :],
                                      yn0[:, SW * j:SW * (j + 1)])
                    nc.sync.dma_start(a2a_in[128 * j + 64:128 * (j + 1), :],
                                      yn1[:, SW * j:SW * (j + 1)])
            if use_collective:
                nc.gpsimd.collective_compute(
                    "AllToAll", mybir.AluOpType.bypass,
                    ins=[a2a_in.opt()], outs=[a2a_out.opt()],
                    replica_groups=[list(range(N_CORES))])
            else:
                # timing-model variant (TimelineSim can't simulate
                # collectives): stand-in DRAM->DRAM copy
                nc.sync.dma_start(a2a_out[:], a2a_in[:])
            for j in range(N_CORES):
                nc.sync.dma_start(a2a_sb[:, j, :],
                                  a2a_out[128 * j:128 * (j + 1), :])

            # ---- phase 4: output projection for this core's SW rows -------
            with tc.tile_pool(name="psC", bufs=2, space="PSUM") as psC:
                mw = min(128, SW)
                for m in range(SW // mw):
                    pp = psC.tile([mw, D_MODEL], f32, tag="pp")
                    for n2 in (0, 1):
                        for k in range(8):
                            nc.tensor.matmul(
                                pp[:, 512 * n2:512 * (n2 + 1)],
                                a2a_sb[:, k, mw * m:mw * (m + 1)],
                                wpj[:, k, 512 * n2:512 * (n2 + 1)],
                                start=(k == 0), stop=(k == 7))
                    ob = stg.tile([mw, D_MODEL], f32, tag="ob", bufs=2)
                    nc.vector.tensor_copy(ob[:], pp[:])
                    nc.sync.dma_start(out_d[mw * m:mw * (m + 1), :], ob[:])

    if split_waits:
        _split_multi_waits(nc)
    return nc


def make_aux_inputs():
    ident = np.eye(128, dtype=BF16)
    k_idx = np.arange(128)[:, None]
    q_idx = np.arange(QB)[None, :]
    masks = np.stack(
        [((k_idx + 128 * d) <= q_idx).astype(BF16) for d in range(4)], axis=0)
    return ident, masks


def make_in_maps(x, w_qkv, w_proj, seq=SEQ):
    x = np.asarray(x, dtype=np.float32).reshape(seq, D_MODEL)
    w_qkv = np.asarray(w_qkv, dtype=np.float32)
    w_proj = np.asarray(w_proj, dtype=np.float32)
    ident, masks = make_aux_inputs()
    in_maps = []
    for i in range(N_CORES):
        sl = slice(CPC * i, CPC * (i + 1))
        w_slice = np.concatenate(
            [w_qkv[:, sl], w_qkv[:, D_MODEL:][:, sl],
             w_qkv[:, 2 * D_MODEL:][:, sl]], axis=1)
        in_maps.append({
            "x": x,
            "w_slice": np.ascontiguousarray(w_slice),
            "w_proj": w_proj,
            "ident": ident,
            "masks": masks,
        })
    return in_maps


_NC_CACHE = {}


def kernel(x, w_qkv, w_proj):
    """Full inputs in, full output out. Shards internally across 8 cores."""
    try:
        import os
        import jax
        jax.config.update("jax_compilation_cache_dir",
                          os.path.expanduser("~/.cache/jax_bass_kernel"))
        jax.config.update("jax_persistent_cache_min_compile_time_secs", 0.0)
    except Exception:
        pass
    from concourse.bass_utils import run_bass_kernel_spmd

    x = np.asarray(x, dtype=np.float32)
    batch = x.shape[0]
    seq = x.shape[1]
    if seq not in _NC_CACHE:
        _NC_CACHE[seq] = build_nc(seq)
    nc = _NC_CACHE[seq]
    in_maps = make_in_maps(x, w_qkv, w_proj, seq=seq)
    res = run_bass_kernel_spmd(nc, in_maps, list(range(N_CORES)))
    out = np.concatenate([res.results[j]["out"] for j in range(N_CORES)],
                         axis=0)
    return out.reshape(batch, seq, D_MODEL).astype(np.float32)



# revision 30
# speedup vs baseline: 1.0970x; 1.0095x over previous
"""Causal self-attention (d_model=1024, n_head=16, seq=4096) on 8 trn2 cores.

Sharding: tensor-parallel over heads (2 heads/core) for QKV + attention,
then an AllToAll re-shards y^T from head-sharded to sequence-sharded, so
each core runs the output projection for seq/8 rows with the full w_proj
(no AllReduce). The host concatenates the 8 row-shards.

Per-core layout (bf16 into the PE, fp32 PSUM accumulation):
  - x^T built via PE identity-matmul transposes (the d_model contraction
    needs x in [c, T] layout for both qkv operands).
  - qkv^T = w_slice.T @ x^T lands directly in [chan, T] layout, so qT/kT
    are exactly the lhsT/rhs of the score matmul (scores^T = K Q^T), and
    V' (normal orientation + a ones column) comes from small PE transposes.
  - softmax without max-subtraction (scores ~ N(0,1): exp cannot overflow
    fp32); the denominator falls out of the y^T matmul as the ones-column
    row; normalization uses exp(-ln(denom)) on ScalarE plus a K=1 matmul
    to broadcast the reciprocal across partitions.
  - causal masking: only lower-triangle k-tiles are computed; diagonal
    tiles are masked by a precomputed 0/1 multiply after the exp.
  - emission is braided: prep for block n+1 (x load/transpose/qkv/V') is
    interleaved between the attention groups of q-block n, under a single
    shared PSUM pool, so PE/ACT/DVE/DMA overlap across phases.
"""

import sys
import types

import numpy as np
import ml_dtypes

D_MODEL = 1024
N_HEAD = 16
SEQ = 4096
N_CORES = 8
D_HEAD = 64
CPC = 128            # channels per core (2 heads x 64)
QB = 512             # attention q-block width
BF16 = ml_dtypes.bfloat16
XBAR_FROM_BLOCK = 99   # blocks >= this: x^T via ACT-queue xbar into scratch
PBC_NORM = True       # normalize broadcast via gpsimd partition_broadcast


def _install_compat_patches():
    """Stub antenv.axon_hooks (absent in this container) so
    run_bass_kernel_spmd's trace path degrades instead of ImportError."""
    if "antenv.axon_hooks" not in sys.modules:
        mod = types.ModuleType("antenv.axon_hooks")
        mod.get_axon_ntff_profile_hook = lambda: None
        sys.modules["antenv.axon_hooks"] = mod


def _split_multi_waits(nc):
    """The nix walrus here accepts at most ONE sync-wait per instruction
    (setupSyncWait: 'Too many sync wait commands').  Hoist extra waits onto
    same-engine NoOps inserted immediately before the instruction — engine
    streams execute in program order, so semantics are unchanged."""
    import concourse.mybir as mybir

    n = 0
    for fn in nc.m.functions:
        for bb in fn.blocks:
            insts = bb.instructions
            out = []
            for inst in insts:
                si = getattr(inst, "sync_info", None)
                waits = list(si.on_wait) if si is not None else []
                if len(waits) > 1:
                    si.on_wait.clear()
                    for w in waits[:-1]:
                        n += 1
                        nop = mybir.InstNoOp(name=f"I-WSPLIT{n}", ins=[], outs=[])
                        nop.engine = inst.engine
                        nop.sync_info = mybir.SyncInfo(on_wait=[w], on_update=[])
                        out.append(nop)
                    si.on_wait.append(waits[-1])
                out.append(inst)
            bb.instructions = out


def build_nc(seq=SEQ, use_collective=True, split_waits=True):
    """Build the single-core SPMD program (identical on all 8 cores)."""
    import concourse.bass as bass
    import concourse.mybir as mybir
    from concourse.tile import TileContext

    _install_compat_patches()

    f32 = mybir.dt.float32
    bf16 = mybir.dt.bfloat16
    AFT = mybir.ActivationFunctionType

    from concourse import library_config

    nT = seq // 128       # T-tiles
    nQB = seq // QB       # attention q-blocks
    SW = seq // N_CORES   # AllToAll shard width (output rows per core)

    nc = bass.Bass("TRN2", target_bir_lowering=False, debug=False,
                   num_devices=N_CORES)
    x_d = nc.dram_tensor("x", [seq, D_MODEL], f32, kind="ExternalInput").ap()
    wq_d = nc.dram_tensor("w_slice", [D_MODEL, 3 * CPC], f32,
                          kind="ExternalInput").ap()
    wp_d = nc.dram_tensor("w_proj", [D_MODEL, D_MODEL], f32,
                          kind="ExternalInput").ap()
    id_d = nc.dram_tensor("ident", [128, 128], bf16, kind="ExternalInput").ap()
    mk_d = nc.dram_tensor("masks", [4, 128, QB], bf16,
                          kind="ExternalInput").ap()
    out_d = nc.dram_tensor("out", [SW, D_MODEL], f32,
                           kind="ExternalOutput").ap()

    with TileContext(nc) as tc:
        with (
            tc.tile_pool(name="per", bufs=1) as per,
            tc.tile_pool(name="stg", bufs=2) as stg,
            tc.tile_pool(name="dram", bufs=1, space="DRAM") as dram,
        ):
            qT = per.tile([128, seq], bf16)      # [2 heads x 64 d, T]
            kT = per.tile([128, seq], bf16)
            Vp = per.tile([128, nT, 130], bf16)  # V' tiles: [v_h0|1|v_h1|1]
            yn0 = per.tile([64, seq], bf16)      # normalized y^T, head 0
            yn1 = per.tile([64, seq], bf16)
            wqkv = per.tile([128, 8, 3 * CPC], bf16)
            wpj = per.tile([128, 8, D_MODEL], bf16)
            iden = per.tile([128, 128], bf16)
            mks = per.tile([128, 4, QB], bf16)
            ones = per.tile([128, 64], f32)
            a2a_sb = per.tile([128, 8, SW], bf16)

            nc.scalar.dma_start(iden[:], id_d[:])
            for m in range(4):
                nc.scalar.dma_start(mks[:, m, :], mk_d[m])
            nc.any.memset(ones[:], 1.0)
            nc.any.memset(Vp[:, :, 64:65], 1.0)
            nc.any.memset(Vp[:, :, 129:130], 1.0)

            # (weight staging happens inside the xstg pool below)

            dnd = dram.tile([16, QB], mybir.dt.float32)
            a2a_in = dram.tile([N_CORES * CPC, SW], bf16)
            a2a_out = dram.tile([N_CORES * CPC, SW], bf16)

            # ---- phases 0-2, braided emission ------------------------
            # Engines execute their scheduled streams in static order, so
            # overlap must be built into emission order: the prep work
            # (x-load/transpose/qkv/V') for block n+1 is interleaved chunk-
            # by-chunk between the attention groups of q-block n.  Attention
            # qb=n depends only on qkv blocks 0..n, so each braid is legal.
            # PSUM banks: pA 2x1 + sT 2x2 + yt0 1 + yt1 1 = 8
            with (
                tc.tile_pool(name="xp", bufs=1) as xp,
                tc.tile_pool(name="xstg", bufs=3) as xstg,
                tc.tile_pool(name="ps", bufs=2, space="PSUM") as ps,
            ):
                xT = xp.tile([128, 8, seq], bf16)   # [c-chunk part, chunk, T]

                def wqkv_stage():
                    for k in range(8):
                        wtmp = xstg.tile([128, 3 * CPC], f32, tag="xf",
                                         bufs=4, name=f"wtmp_{k}")
                        nc.sync.dma_start(wtmp[:],
                                          wq_d[128 * k:128 * (k + 1), :])
                        nc.vector.tensor_copy(wqkv[:, k, :], wtmp[:])

                def prep_chunks(n):
                    """Emit-closures for block n: loads, x^T, qkv^T, V'."""
                    state = {}

                    def loads():
                        xbs = []
                        for u in range(4):
                            t = 4 * n + u
                            xf = xstg.tile([128, D_MODEL], f32, tag="xf",
                                           bufs=4, name=f"xf_{t}")
                            nc.sync.dma_start(xf[:],
                                              x_d[128 * t:128 * (t + 1), :])
                            xb = xstg.tile([128, D_MODEL], bf16, tag="xb",
                                           bufs=6, name=f"xb_{t}")
                            nc.gpsimd.tensor_copy(xb[:], xf[:])
                            xbs.append(xb)
                        state["xbs"] = xbs

                    def xtr(u):
                        def emit():
                            t = 4 * n + u
                            xc = xstg.tile([128, 8, 128], bf16, tag="xc",
                                           bufs=3, name=f"xc_{t}")
                            nc.scalar.dma_start_transpose(
                                xc[:], state["xbs"][u][:])
                            nc.vector.tensor_copy(
                                xT[:, :, 128 * t:128 * (t + 1)], xc[:])
                        return emit

                    def trans(j):
                        # j indexes (x-tile u = j//2, c-chunk quad a = j%2):
                        # one PSUM tile holds 4 c-chunk transposes of a
                        # single x-tile, so work starts after its one load
                        def emit():
                            u, a = divmod(j, 2)
                            tp = ps.tile([128, 512], f32, tag="pA",
                                         name=f"tp_{n}_{j}")
                            for c in range(4):
                                nc.tensor.matmul(
                                    tp[:, 128 * c:128 * (c + 1)],
                                    state["xbs"][u][:, 128 * (4 * a + c):
                                                    128 * (4 * a + c + 1)],
                                    iden[:], start=True, stop=True)
                            cpe = nc.vector if (j % 2 == 0) else nc.scalar
                            if cpe is nc.vector:
                                cpe.tensor_copy(
                                    xT[:, 4 * a:4 * (a + 1),
                                       128 * (4 * n + u):
                                       128 * (4 * n + u + 1)],
                                    tp[:])
                            else:
                                nc.scalar.copy(
                                    xT[:, 4 * a:4 * (a + 1),
                                       128 * (4 * n + u):
                                       128 * (4 * n + u + 1)],
                                    tp[:])
                        return emit

                    def qkv(m):
                        def emit():
                            qp = ps.tile([128, 512], f32, tag="pA",
                                         name=f"qp_{n}_{m}")
                            for k in range(8):
                                nc.tensor.matmul(
                                    qp[:],
                                    wqkv[:, k, 128 * m:128 * (m + 1)],
                                    xT[:, k, 512 * n:512 * (n + 1)],
                                    start=(k == 0), stop=(k == 7))
                            if m == 0:
                                nc.scalar.copy(
                                    qT[:, 512 * n:512 * (n + 1)], qp[:])
                            elif m == 1:
                                nc.scalar.copy(
                                    kT[:, 512 * n:512 * (n + 1)], qp[:])
                            else:
                                vs = xstg.tile([128, 512], bf16, tag="vs",
                                               bufs=2, name=f"vs_{n}")
                                nc.vector.tensor_copy(vs[:], qp[:])
                                state["vs"] = vs
                        return emit

                    def vtr(u):
                        def emit():
                            t = 4 * n + u
                            vs = state["vs"]
                            # separate PSUM tiles per head: PE-write plus
                            # DVE-read of one PSUM bank is a HW fault
                            vp0 = ps.tile([128, 64], f32, tag="pA",
                                          name=f"vp0_{t}")
                            vp1 = ps.tile([128, 64], f32, tag="pA",
                                          name=f"vp1_{t}")
                            nc.tensor.matmul(
                                vp0[:], vs[0:64, 128 * u:128 * (u + 1)],
                                iden[0:64, 0:64], start=True, stop=True)
                            nc.tensor.matmul(
                                vp1[:], vs[64:128, 128 * u:128 * (u + 1)],
                                iden[64:128, 64:128], start=True, stop=True)
                            nc.vector.tensor_copy(Vp[:, t, 0:64], vp0[:])
                            nc.vector.tensor_copy(Vp[:, t, 65:129], vp1[:])
                        return emit

                    if n >= XBAR_FROM_BLOCK:
                        return ([loads] + [xtr(u) for u in range(4)]
                                + [qkv(m) for m in range(3)]
                                + [vtr(u) for u in range(4)])
                    return ([loads] + [trans(j) for j in range(8)]
                            + [qkv(m) for m in range(3)]
                            + [vtr(u) for u in range(4)])

                def attention_groups(qb, ytps):
                    nkt = 4 * (qb + 1)

                    def group(g):
                        # diagonal k-tiles (d = kt-4qb >= 0) only attend to
                        # q >= 128d: trim score MM / exp / mask / yT MM to
                        # the valid column range [128d, QB).  q-cols below
                        # that are fully masked and, because kt=0 always
                        # covers the full width with start=True, never read.
                        def off(kt):
                            d = kt - 4 * qb
                            return 128 * d if d >= 0 else 0

                        def emit():
                            # h-inner MM order: consecutive score matmuls use
                            # disjoint PE row-groups (h0 rows 0-63, h1 rows
                            # 64-127) so the 16x32x32-subarray PE overlaps
                            # them (K=64 packing, ~2x on the score matmuls)
                            sps = [ps.tile([128, 2 * QB], f32, tag="sT",
                                           name=f"sp_{qb}_{g}_{h}")
                                   for h in (0, 1)]
                            for u in (0, 1):
                                kt = 2 * g + u
                                o = off(kt)
                                for h in (0, 1):
                                    nc.tensor.matmul(
                                        sps[h][:, QB * u + o:QB * (u + 1)],
                                        kT[64 * h:64 * (h + 1),
                                           128 * kt:128 * (kt + 1)],
                                        qT[64 * h:64 * (h + 1),
                                           QB * qb + o:QB * (qb + 1)],
                                        start=True, stop=True)
                            diag = off(2 * g) > 0 or off(2 * g + 1) > 0
                            for h in (0, 1):
                                pt = stg.tile([128, 2 * QB], bf16, tag="pT",
                                              bufs=3, name=f"pt_{qb}_{g}_{h}")
                                if diag:
                                    for u in (0, 1):
                                        o = off(2 * g + u)
                                        nc.scalar.activation(
                                            pt[:, QB * u + o:QB * (u + 1)],
                                            sps[h][:, QB * u + o:QB * (u + 1)],
                                            AFT.Exp, scale=0.125)
                                else:
                                    nc.scalar.activation(pt[:], sps[h][:],
                                                         AFT.Exp, scale=0.125)
                                for u in (0, 1):
                                    kt = 2 * g + u
                                    d = kt - 4 * qb
                                    o = off(kt)
                                    if d >= 0:
                                        w = min(o + 128, QB)
                                        nc.vector.tensor_mul(
                                            pt[:, QB * u + o:QB * u + w],
                                            pt[:, QB * u + o:QB * u + w],
                                            mks[:, d, o:w])
                                    nc.tensor.matmul(
                                        ytps[h][:, o:QB],
                                        Vp[:, kt, 65 * h:65 * (h + 1)],
                                        pt[:, QB * u + o:QB * (u + 1)],
                                        start=(kt == 0),
                                        stop=(kt == nkt - 1))
                        return emit

                    return [group(g) for g in range(nkt // 2)]

                def normalize(qb, ytps):
                    for h in (0, 1):
                        # one copy frees the PSUM accumulator right away; the
                        # denom -> 1/denom -> broadcast -> scale chain then
                        # runs from SBUF off the critical path.
                        yu = stg.tile([65, 2 * QB], f32, tag="dn", bufs=4,
                                      name=f"yu_{qb}_{h}")
                        nc.vector.tensor_copy(yu[:, 0:QB], ytps[h][:])
                        nc.vector.reciprocal(yu[64:65, QB:2 * QB],
                                             yu[64:65, 0:QB])
                        yn = yn0 if h == 0 else yn1
                        if PBC_NORM and qb < nQB - 1:
                            bcs = stg.tile([64, QB], f32, tag="bc2", bufs=2,
                                           name=f"bcs_{qb}_{h}")
                            dslot = dnd[2 * qb + h:2 * qb + h + 1, :]
                            nc.sync.dma_start(dslot, yu[64:65, QB:2 * QB])
                            nc.sync.dma_start(
                                bcs[:], dslot.broadcast_to([64, QB]))
                            nc.vector.tensor_mul(
                                yn[:, QB * qb:QB * (qb + 1)],
                                yu[0:64, 0:QB], bcs[:])
                        else:
                            bcp = ps.tile([64, QB], f32, tag="pA",
                                          name=f"bcp_{qb}_{h}")
                            nc.tensor.matmul(bcp[:], ones[64:65, 0:64],
                                             yu[64:65, QB:2 * QB],
                                             start=True, stop=True)
                            nc.vector.tensor_mul(
                                yn[:, QB * qb:QB * (qb + 1)],
                                yu[0:64, 0:QB], bcp[:])
                        if SW == QB:
                            # q-block == shard: stage its AllToAll rows now
                            j = qb
                            r0 = 128 * j + 64 * h
                            nc.sync.dma_start(a2a_in[r0:r0 + 64, :],
                                              yn[:, SW * j:SW * (j + 1)])

                def wpj_chunk(k):
                    def emit():
                        # w_proj staged late (projection tail only) and
                        # braided into the final attention block, which has
                        # no other prep work to overlap with
                        ptmp = xstg.tile([128, D_MODEL], f32, tag="xf",
                                         bufs=4, name=f"ptmp_{k}")
                        nc.sync.dma_start(ptmp[:],
                                          wp_d[128 * k:128 * (k + 1), :])
                        nc.gpsimd.tensor_copy(wpj[:, k, :], ptmp[:])
                    return emit

                p0 = prep_chunks(0)
                p0[0]()           # stage-0 x loads lead the DMA queues
                wqkv_stage()
                for c in p0[1:]:
                    c()
                for n in range(nQB):
                    ytps = [ps.tile([65, QB], f32, tag=f"yt{h}", bufs=1,
                                    name=f"yt{h}_{n}") for h in (0, 1)]
                    if n + 1 < nQB:
                        pend = prep_chunks(n + 1)
                    else:
                        pend = [wpj_chunk(k) for k in range(8)]
                    groups = attention_groups(n, ytps)
                    ci = 0
                    for gi, g in enumerate(groups):
                        g()
                        want = (gi + 1) * len(pend) // len(groups)
                        while ci < want:
                            pend[ci]()
                            ci += 1
                    while ci < len(pend):
                        pend[ci]()
                        ci += 1
                    normalize(n, ytps)

            # ---- phase 3: AllToAll head-shard -> seq-shard ----------------
            if SW != QB:
                for j in range(N_CORES):
                    nc.sync.dma_start(a2a_in[128 * j:128 * j + 64, :],
                                      yn0[:, SW * j:SW * (j + 1)])
                    nc.sync.dma_start(a2a_in[128 * j + 64:128 * (j + 1), :],
                                      yn1[:, SW * j:SW * (j + 1)])
            if use_collective:
                nc.gpsimd.collective_compute(
                    "AllToAll", mybir.AluOpType.bypass,
                    ins=[a2a_in.opt()], outs=[a2a_out.opt()],
                    replica_groups=[list(range(N_CORES))])
            else:
                # timing-model variant (TimelineSim can't simulate
                # collectives): stand-in DRAM->DRAM copy
                nc.sync.dma_start(a2a_out[:], a2a_in[:])
            for j in range(N_CORES):
                nc.sync.dma_start(a2a_sb[:, j, :],
                                  a2a_out[128 * j:128 * (j + 1), :])

            # ---- phase 4: output projection for this core's SW rows -------
            with tc.tile_pool(name="psC", bufs=2, space="PSUM") as psC:
                mw = min(128, SW)
                for m in range(SW // mw):
                    pp = psC.tile([mw, D_MODEL], f32, tag="pp")
                    for n2 in (0, 1):
                        for k in range(8):
                            nc.tensor.matmul(
                                pp[:, 512 * n2:512 * (n2 + 1)],
                                a2a_sb[:, k, mw * m:mw * (m + 1)],
                                wpj[:, k, 512 * n2:512 * (n2 + 1)],
                                start=(k == 0), stop=(k == 7))
                    ob = stg.tile([mw, D_MODEL], f32, tag="ob", bufs=2)
                    nc.vector.tensor_copy(ob[:], pp[:])
                    nc.sync.dma_start(out_d[mw * m:mw * (m + 1), :], ob[:])

    if split_waits:
        _split_multi_waits(nc)
    return nc


def make_aux_inputs():
    ident = np.eye(128, dtype=BF16)
    k_idx = np.arange(128)[:, None]
    q_idx = np.arange(QB)[None, :]
    masks = np.stack(
        [((k_idx + 128 * d) <= q_idx).astype(BF16) for d in range(4)], axis=0)
    return ident, masks


def make_in_maps(x, w_qkv, w_proj, seq=SEQ):
    x = np.asarray(x, dtype=np.float32).reshape(seq, D_MODEL)
    w_qkv = np.asarray(w_qkv, dtype=np.float32)
    w_proj = np.asarray(w_proj, dtype=np.float32)
    ident, masks = make_aux_inputs()
    in_maps = []
    for i in range(N_CORES):
        sl = slice(CPC * i, CPC * (i + 1))
        w_slice = np.concatenate(
            [w_qkv[:, sl], w_qkv[:, D_MODEL:][:, sl],
             w_qkv[:, 2 * D_MODEL:][:, sl]], axis=1)
        in_maps.append({
            "x": x,
            "w_slice": np.ascontiguousarray(w_slice),
            "w_proj": w_proj,
            "ident": ident,
            "masks": masks,
        })
    return in_maps


_NC_CACHE = {}


def kernel(x, w_qkv, w_proj):
    """Full inputs in, full output out. Shards internally across 8 cores."""
    try:
        import os
        import jax
        jax.config.update("jax_compilation_cache_dir",
                          os.path.expanduser("~/.cache/jax_bass_kernel"))
        jax.config.update("jax_persistent_cache_min_compile_time_secs", 0.0)
    except Exception:
        pass
    from concourse.bass_utils import run_bass_kernel_spmd

    x = np.asarray(x, dtype=np.float32)
    batch = x.shape[0]
    seq = x.shape[1]
    if seq not in _NC_CACHE:
        _NC_CACHE[seq] = build_nc(seq)
    nc = _NC_CACHE[seq]
    in_maps = make_in_maps(x, w_qkv, w_proj, seq=seq)
    res = run_bass_kernel_spmd(nc, in_maps, list(range(N_CORES)))
    out = np.concatenate([res.results[j]["out"] for j in range(N_CORES)],
                         axis=0)
    return out.reshape(batch, seq, D_MODEL).astype(np.float32)

